# revision 5
# baseline (speedup 1.0000x reference)
"""MoE layer (8 experts, top-2 routing, SwiGLU FFN) for 8 Trainium2 NeuronCores.

Sharding strategy (expert-parallel with host-side token dispatch):
  - The router (x @ gate_w.T, top-2, softmax) runs on host as part of computing
    the token dispatch = the sharding of work across cores.
  - Core e receives only the tokens routed to expert e (gathered, padded to a
    common capacity C) plus expert e's weights, all pre-transposed and cast to
    bf16 on host for the device matmul layout.
  - The device kernel computes the expert SwiGLU FFN:
        out = rw * ((silu(x @ w1.T) * (x @ w3.T)) @ w2.T)
    entirely out of SBUF-resident operands (bf16 matmuls, fp32 accumulation).
  - Host scatter-adds the per-expert outputs back into the full (B,S,d) output
    (top-2 => each token's output is the sum of two expert contributions).
  - The auxiliary load-balance loss is a cheap scalar reduction done on host.
"""

import numpy as np
import ml_dtypes

import concourse.bass as bass
import concourse.tile as tile
import concourse.mybir as mybir
from concourse.bass_utils import run_bass_kernel_spmd
from concourse.vector_clock import ScopedClock

BF16 = ml_dtypes.bfloat16
AFT = mybir.ActivationFunctionType

TOP_K = 2
NUM_EXPERTS = 8
D_MODEL = 1024
D_FF = 2048
N_CORES = 8
P = 128

_PATCHED = False


def _patch_drain_wait_split():
    """This walrus build caps sync waits at 1 per instruction (2 for EVSEM),
    but TileContext's final drain can carry one wait per outstanding engine /
    DMA queue.  Split them across individual single-wait sync nops."""
    global _PATCHED
    if _PATCHED:
        return

    def _split_drain_and_barrier(self, tick_clock, wait_clock):
        probe = self.nc.sync.drain()
        wait_clock.add_sem_waits(
            probe.ins, ScopedClock({None: tick_clock.global_clock})
        )
        si = probe.ins.sync_info
        if si is not None and len(si.on_wait) > 1:
            waits = list(si.on_wait)
            probe.ins.sync_info = mybir.SyncInfo(
                on_wait=[waits[0]], on_update=list(si.on_update)
            )
            for w in waits[1:]:
                extra = self.nc.sync.nop(nofuse=True)
                extra.ins.sync_info = mybir.SyncInfo(on_wait=[w], on_update=[])
        self.nc.all_engine_barrier()
        assert self.sems is not None
        popped = self.nc._tile_sem_poison_stack.pop()
        assert popped is self._sem_poison
        self.nc.clear_and_free_semaphores(list(self.sems.allocated().values()))
        self.nc.all_engine_barrier()

    tile.TileContext._drain_and_barrier = _split_drain_and_barrier
    _PATCHED = True


def _legalize_waits(nc):
    """Hardware wait-slot cap: 1 sync wait per instruction (2 for EVSEM).
    Tile's sem assignment can emit more; hoist extras onto single-wait nops
    inserted just before the instruction on the same engine (engines process
    their streams in order, so this preserves semantics)."""
    for fn in nc.m.functions:
        for bb in fn.blocks:
            insts = bb.instructions  # live list
            i = 0
            while i < len(insts):
                inst = insts[i]
                si = inst.sync_info
                cap = 2 if isinstance(inst, mybir.InstEventSemaphore) else 1
                if si is not None and len(si.on_wait) > cap:
                    waits = list(si.on_wait)
                    inst.sync_info = mybir.SyncInfo(
                        on_wait=waits[:cap], on_update=list(si.on_update)
                    )
                    for j, w in enumerate(waits[cap:]):
                        nop = mybir.InstNoOp(
                            name=f"{inst.name}-waitsplit-{j}",
                            sync_info=mybir.SyncInfo(on_wait=[w], on_update=[]),
                            bass_nofuse=True,
                            engine=inst.engine,
                        )
                        insts.insert(i, nop)
                        i += 1
                i += 1


def build_ffn(C):
    """Per-core SwiGLU expert FFN over C gathered tokens.

    DRAM inputs (all bf16 except rw):
      xT  [D_MODEL, C] : gathered tokens, transposed (d on rows)
      w1T [D_MODEL, D_FF], w3T [D_MODEL, D_FF], w2T [D_FF, D_MODEL]
      rw  [C] fp32     : per-token routing weight (0 for padding)
    DRAM output:
      out [C, D_MODEL] fp32 : rw-scaled expert output rows
    """
    _patch_drain_wait_split()
    nc = bass.Bass()
    DT = mybir.dt.bfloat16
    KD = D_MODEL // P   # 8 contraction chunks over d_model
    KF = D_FF // P      # 16 contraction chunks over d_ff
    MB = C // P         # token blocks of 128

    xT = nc.dram_tensor("xT", [D_MODEL, C], DT, kind="ExternalInput")
    w1T = nc.dram_tensor("w1T", [D_MODEL, D_FF], DT, kind="ExternalInput")
    w3T = nc.dram_tensor("w3T", [D_MODEL, D_FF], DT, kind="ExternalInput")
    w2T = nc.dram_tensor("w2T", [D_FF, D_MODEL], DT, kind="ExternalInput")
    rw = nc.dram_tensor("rw", [C], mybir.dt.float32, kind="ExternalInput")
    out = nc.dram_tensor("out", [C, D_MODEL], mybir.dt.float32, kind="ExternalOutput")

    with tile.TileContext(nc) as tc:
        with (
            tc.tile_pool(name="weights", bufs=1) as wpool,
            tc.tile_pool(name="acts", bufs=1) as apool,
            tc.tile_pool(name="tmp", bufs=4) as tpool,
            tc.tile_pool(name="outp", bufs=4) as opool,
            tc.tile_pool(name="psum", bufs=2, space="PSUM") as psum,
        ):
            # SBUF-resident operands
            x_sb = apool.tile([P, KD, C], DT, tag="x")
            y_sb = apool.tile([P, KF, C], DT, tag="y")
            rw_sb = apool.tile([P, MB], mybir.dt.float32, tag="rw")
            w1_sb = wpool.tile([P, KD, D_FF], DT, tag="w1")
            w3_sb = wpool.tile([P, KD, D_FF], DT, tag="w3")
            w2_sb = wpool.tile([P, KF, D_MODEL], DT, tag="w2")

            xT_r = xT.rearrange("(ko p) c -> p ko c", p=P)
            w1_r = w1T.rearrange("(ko p) f -> p ko f", p=P)
            w3_r = w3T.rearrange("(ko p) f -> p ko f", p=P)
            w2_r = w2T.rearrange("(ko p) d -> p ko d", p=P)

            # token blocks of up to 512 (one PSUM bank of fp32)
            tbs = []
            off = 0
            while off < C:
                sz = min(512, C - off)
                tbs.append((off, sz))
                off += sz

            # DMA issue order follows consumption order: the first phase-A
            # groups need x[:, tb0] and the first d_ff quarter of w1/w3;
            # later quarters, later token blocks, then w2 stream in under
            # compute.  Fine granularity lets matmuls start early.
            t0off, t0sz = tbs[0]
            for k in range(KD):
                nc.sync.dma_start(
                    x_sb[:, k, t0off : t0off + t0sz], xT_r[:, k, t0off : t0off + t0sz]
                )
            FQ = D_FF // 4
            for q in range(4):
                for k in range(KD):
                    nc.sync.dma_start(
                        w1_sb[:, k, q * FQ : (q + 1) * FQ],
                        w1_r[:, k, q * FQ : (q + 1) * FQ],
                    )
                    nc.sync.dma_start(
                        w3_sb[:, k, q * FQ : (q + 1) * FQ],
                        w3_r[:, k, q * FQ : (q + 1) * FQ],
                    )
            for toff, tsz in tbs[1:]:
                for k in range(KD):
                    nc.sync.dma_start(
                        x_sb[:, k, toff : toff + tsz], xT_r[:, k, toff : toff + tsz]
                    )
            for k in range(KF):
                nc.sync.dma_start(w2_sb[:, k], w2_r[:, k])
            nc.sync.dma_start(rw_sb[:], rw.rearrange("(mb p) -> p mb", p=P))

            # PE warm-up: ~8 dummy matmuls on memset tiles flip the HAM clock
            # gate to 8/8 while the first DMAs are still in flight.
            wa = tpool.tile([P, P], DT, tag="warm_a", name="wa")
            wb = tpool.tile([P, 512], DT, tag="warm_b", name="wb")
            nc.vector.memset(wa[:], 0.0)
            nc.vector.memset(wb[:], 0.0)
            pw = psum.tile([P, 512], mybir.dt.float32, tag="pw", name="pw")
            for _ in range(8):
                nc.tensor.matmul(pw, lhsT=wa[:], rhs=wb[:], start=True, stop=True)

            for toff, tsz in tbs:
                # Phase A: hT/vT = w1/w3 @ x for this token block, all d_ff rows
                for fb in range(KF):
                    ph = psum.tile([P, 512], mybir.dt.float32, tag="ph", name="ph")[:, :tsz]
                    pv = psum.tile([P, 512], mybir.dt.float32, tag="pv", name="pv")[:, :tsz]
                    for k in range(KD):
                        nc.tensor.matmul(
                            ph,
                            lhsT=w1_sb[:, k, fb * P : (fb + 1) * P],
                            rhs=x_sb[:, k, toff : toff + tsz],
                            start=(k == 0),
                            stop=(k == KD - 1),
                        )
                    for k in range(KD):
                        nc.tensor.matmul(
                            pv,
                            lhsT=w3_sb[:, k, fb * P : (fb + 1) * P],
                            rhs=x_sb[:, k, toff : toff + tsz],
                            start=(k == 0),
                            stop=(k == KD - 1),
                        )
                    sil = tpool.tile([P, 512], mybir.dt.float32, tag="sil", name="sil")[:, :tsz]
                    nc.scalar.activation(sil, ph, AFT.Silu)
                    nc.vector.tensor_mul(
                        out=y_sb[:, fb, toff : toff + tsz], in0=sil, in1=pv
                    )
                # Phase B: out = y @ w2 for this token block (tokens on partitions)
                for mi in range(tsz // P):
                    mb = toff // P + mi
                    for db in range(D_MODEL // 512):
                        po = psum.tile([P, 512], mybir.dt.float32, tag="po", name="po")
                        for kf in range(KF):
                            nc.tensor.matmul(
                                po,
                                lhsT=y_sb[:, kf, mb * P : (mb + 1) * P],
                                rhs=w2_sb[:, kf, db * 512 : (db + 1) * 512],
                                start=(kf == 0),
                                stop=(kf == KF - 1),
                            )
                        ob = opool.tile([P, 512], mybir.dt.float32, tag="ob", name="ob")
                        nc.vector.tensor_scalar_mul(ob[:], po, rw_sb[:, mb : mb + 1])
                        nc.sync.dma_start(
                            out[mb * P : (mb + 1) * P, db * 512 : (db + 1) * 512], ob
                        )
    _legalize_waits(nc)
    return nc


_BUILD_CACHE = {}


def _get_nc(C):
    if C not in _BUILD_CACHE:
        _BUILD_CACHE[C] = build_ffn(C)
    return _BUILD_CACHE[C]


def _route(xf, gate_w):
    """Top-2 routing (matches jax.lax.top_k + softmax in fp32)."""
    T = xf.shape[0]
    logits = xf @ gate_w.T  # (T, E) fp32
    i1 = np.argmax(logits, axis=1)
    l1 = logits[np.arange(T), i1]
    masked = logits.copy()
    masked[np.arange(T), i1] = -np.inf
    i2 = np.argmax(masked, axis=1)
    l2 = logits[np.arange(T), i2]
    e2 = np.exp((l2 - l1).astype(np.float32))
    rw1 = (1.0 / (1.0 + e2)).astype(np.float32)
    rw2 = (e2 / (1.0 + e2)).astype(np.float32)
    return logits, i1, i2, rw1, rw2


def kernel(x, gate_w, w1, w3, w2, _run_opts=None):
    x = np.ascontiguousarray(np.asarray(x, dtype=np.float32))
    gate_w = np.asarray(gate_w, dtype=np.float32)
    w1 = np.asarray(w1, dtype=np.float32)
    w3 = np.asarray(w3, dtype=np.float32)
    w2 = np.asarray(w2, dtype=np.float32)

    B, S, d = x.shape
    T = B * S
    E = NUM_EXPERTS
    xf = x.reshape(T, d)

    logits, i1, i2, rw1, rw2 = _route(xf, gate_w)

    # Per-expert token lists + shared capacity (multiple of 128)
    sels, rws = [], []
    for e in range(E):
        sel = np.where((i1 == e) | (i2 == e))[0]
        sels.append(sel)
        rws.append(np.where(i1[sel] == e, rw1[sel], rw2[sel]).astype(np.float32))
    C = max(256, -(-max(len(s) for s in sels) // P) * P)

    nc = _get_nc(C)

    in_maps = []
    for e in range(E):
        sel = sels[e]
        n = len(sel)
        xTe = np.zeros((d, C), dtype=BF16)
        xTe[:, :n] = np.ascontiguousarray(xf[sel].T).astype(BF16)
        rwe = np.zeros((C,), dtype=np.float32)
        rwe[:n] = rws[e]
        in_maps.append(
            {
                "xT": xTe,
                "w1T": np.ascontiguousarray(w1[e].T).astype(BF16),
                "w3T": np.ascontiguousarray(w3[e].T).astype(BF16),
                "w2T": np.ascontiguousarray(w2[e].T).astype(BF16),
                "rw": rwe,
            }
        )

    run_opts = _run_opts or {}
    res = run_bass_kernel_spmd(nc, in_maps, core_ids=list(range(N_CORES)), **run_opts)

    outf = np.zeros((T, d), dtype=np.float32)
    for e in range(E):
        sel = sels[e]
        outf[sel] += res.results[e]["out"][: len(sel)]
    output = outf.reshape(B, S, d)

    # Auxiliary load-balance loss (host, fp32 scalar)
    probs = np.exp(logits - logits.max(axis=1, keepdims=True))
    probs /= probs.sum(axis=1, keepdims=True)
    counts = np.bincount(np.concatenate([i1, i2]), minlength=E)
    aux = np.float32(
        (probs.mean(axis=0) * (counts / (T * TOP_K))).sum() * E
    )

    if _run_opts is not None:
        return (output, aux), res
    return output, aux


# revision 7
# speedup vs baseline: 1.0360x; 1.0360x over previous
"""MoE layer (8 experts, top-2 routing, SwiGLU FFN) for 8 Trainium2 NeuronCores.

Sharding strategy (expert-parallel with host-side token dispatch):
  - The router (x @ gate_w.T, top-2, softmax) runs on host as part of computing
    the token dispatch = the sharding of work across cores.
  - Core e receives only the tokens routed to expert e (gathered, padded to a
    common capacity C) plus expert e's weights, all pre-transposed and cast to
    bf16 on host for the device matmul layout.
  - The device kernel computes the expert SwiGLU FFN:
        out = rw * ((silu(x @ w1.T) * (x @ w3.T)) @ w2.T)
    entirely out of SBUF-resident operands (bf16 matmuls, fp32 accumulation).
  - Host scatter-adds the per-expert outputs back into the full (B,S,d) output
    (top-2 => each token's output is the sum of two expert contributions).
  - The auxiliary load-balance loss is a cheap scalar reduction done on host.
"""

import numpy as np
import ml_dtypes

import concourse.bass as bass
import concourse.tile as tile
import concourse.mybir as mybir
from concourse.bass_utils import run_bass_kernel_spmd
from concourse.vector_clock import ScopedClock

BF16 = ml_dtypes.bfloat16
AFT = mybir.ActivationFunctionType

TOP_K = 2
NUM_EXPERTS = 8
D_MODEL = 1024
D_FF = 2048
N_CORES = 8
P = 128

_PATCHED = False


def _patch_drain_wait_split():
    """This walrus build caps sync waits at 1 per instruction (2 for EVSEM),
    but TileContext's final drain can carry one wait per outstanding engine /
    DMA queue.  Split them across individual single-wait sync nops."""
    global _PATCHED
    if _PATCHED:
        return

    def _split_drain_and_barrier(self, tick_clock, wait_clock):
        probe = self.nc.sync.drain()
        wait_clock.add_sem_waits(
            probe.ins, ScopedClock({None: tick_clock.global_clock})
        )
        si = probe.ins.sync_info
        if si is not None and len(si.on_wait) > 1:
            waits = list(si.on_wait)
            probe.ins.sync_info = mybir.SyncInfo(
                on_wait=[waits[0]], on_update=list(si.on_update)
            )
            for w in waits[1:]:
                extra = self.nc.sync.nop(nofuse=True)
                extra.ins.sync_info = mybir.SyncInfo(on_wait=[w], on_update=[])
        self.nc.all_engine_barrier()
        assert self.sems is not None
        popped = self.nc._tile_sem_poison_stack.pop()
        assert popped is self._sem_poison
        self.nc.clear_and_free_semaphores(list(self.sems.allocated().values()))
        self.nc.all_engine_barrier()

    tile.TileContext._drain_and_barrier = _split_drain_and_barrier
    _PATCHED = True


def _legalize_waits(nc):
    """Hardware wait-slot cap: 1 sync wait per instruction (2 for EVSEM).
    Tile's sem assignment can emit more; hoist extras onto single-wait nops
    inserted just before the instruction on the same engine (engines process
    their streams in order, so this preserves semantics)."""
    for fn in nc.m.functions:
        for bb in fn.blocks:
            insts = bb.instructions  # live list
            i = 0
            while i < len(insts):
                inst = insts[i]
                si = inst.sync_info
                cap = 2 if isinstance(inst, mybir.InstEventSemaphore) else 1
                if si is not None and len(si.on_wait) > cap:
                    waits = list(si.on_wait)
                    inst.sync_info = mybir.SyncInfo(
                        on_wait=waits[:cap], on_update=list(si.on_update)
                    )
                    for j, w in enumerate(waits[cap:]):
                        nop = mybir.InstNoOp(
                            name=f"{inst.name}-waitsplit-{j}",
                            sync_info=mybir.SyncInfo(on_wait=[w], on_update=[]),
                            bass_nofuse=True,
                            engine=inst.engine,
                        )
                        insts.insert(i, nop)
                        i += 1
                i += 1


def build_ffn(C):
    """Per-core SwiGLU expert FFN over C gathered tokens.

    DRAM inputs (all bf16 except rw):
      xT  [D_MODEL, C] : gathered tokens, transposed (d on rows)
      w1T [D_MODEL, D_FF], w3T [D_MODEL, D_FF], w2T [D_FF, D_MODEL]
      rw  [C] fp32     : per-token routing weight (0 for padding)
    DRAM output:
      out [C, D_MODEL] fp32 : rw-scaled expert output rows
    """
    _patch_drain_wait_split()
    nc = bass.Bass()
    DT = mybir.dt.bfloat16
    KD = D_MODEL // P   # 8 contraction chunks over d_model
    KF = D_FF // P      # 16 contraction chunks over d_ff
    MB = C // P         # token blocks of 128

    xT = nc.dram_tensor("xT", [D_MODEL, C], DT, kind="ExternalInput")
    w1T = nc.dram_tensor("w1T", [D_MODEL, D_FF], DT, kind="ExternalInput")
    w3T = nc.dram_tensor("w3T", [D_MODEL, D_FF], DT, kind="ExternalInput")
    w2T = nc.dram_tensor("w2T", [D_FF, D_MODEL], DT, kind="ExternalInput")
    rw = nc.dram_tensor("rw", [C], mybir.dt.float32, kind="ExternalInput")
    out = nc.dram_tensor("out", [C, D_MODEL], mybir.dt.float32, kind="ExternalOutput")

    with tile.TileContext(nc) as tc:
        with (
            tc.tile_pool(name="weights", bufs=1) as wpool,
            tc.tile_pool(name="acts", bufs=1) as apool,
            tc.tile_pool(name="tmp", bufs=4) as tpool,
            tc.tile_pool(name="outp", bufs=4) as opool,
            tc.tile_pool(name="psum", bufs=2, space="PSUM") as psum,
        ):
            # SBUF-resident operands
            x_sb = apool.tile([P, KD, C], DT, tag="x")
            y_sb = apool.tile([P, KF, C], DT, tag="y")
            rw_sb = apool.tile([P, MB], mybir.dt.float32, tag="rw")
            w1_sb = wpool.tile([P, KD, D_FF], DT, tag="w1")
            w3_sb = wpool.tile([P, KD, D_FF], DT, tag="w3")
            w2_sb = wpool.tile([P, KF, D_MODEL], DT, tag="w2")

            xT_r = xT.rearrange("(ko p) c -> p ko c", p=P)
            w1_r = w1T.rearrange("(ko p) f -> p ko f", p=P)
            w3_r = w3T.rearrange("(ko p) f -> p ko f", p=P)
            w2_r = w2T.rearrange("(ko p) d -> p ko d", p=P)

            # token blocks of up to 512 (one PSUM bank of fp32)
            tbs = []
            off = 0
            while off < C:
                sz = min(512, C - off)
                tbs.append((off, sz))
                off += sz

            # DMA waves ordered by consumption: HWDGE queues run concurrently
            # and share HBM bandwidth, so later waves are explicitly gated on
            # earlier ones (add_dep_helper) — the critical first tiles get the
            # full bandwidth and matmuls start ~6us in.
            from concourse.tile_rust import add_dep_helper

            t0off, t0sz = tbs[0]
            waves = []
            waves.append(
                [
                    nc.sync.dma_start(
                        x_sb[:, :, t0off : t0off + t0sz],
                        xT_r[:, :, t0off : t0off + t0sz],
                    ),
                    nc.sync.dma_start(w1_sb[:, :, 0:256], w1_r[:, :, 0:256]),
                    nc.sync.dma_start(w3_sb[:, :, 0:256], w3_r[:, :, 0:256]),
                ]
            )
            for lo, hi in ((256, 768), (768, 1280), (1280, 2048)):
                waves.append(
                    [
                        nc.sync.dma_start(w1_sb[:, :, lo:hi], w1_r[:, :, lo:hi]),
                        nc.sync.dma_start(w3_sb[:, :, lo:hi], w3_r[:, :, lo:hi]),
                    ]
                )
            rest = [
                nc.sync.dma_start(
                    x_sb[:, :, t0off + t0sz :], xT_r[:, :, t0off + t0sz :]
                ),
                nc.sync.dma_start(rw_sb[:], rw.rearrange("(mb p) -> p mb", p=P)),
            ]
            waves.append(rest)
            waves.append(
                [
                    nc.sync.dma_start(w2_sb[:, :KF // 2], w2_r[:, :KF // 2]),
                    nc.sync.dma_start(w2_sb[:, KF // 2 :], w2_r[:, KF // 2 :]),
                ]
            )
            for prev, nxt in zip(waves, waves[1:]):
                for d in nxt:
                    for p_ in prev:
                        add_dep_helper(d.ins, p_.ins, True, "dma wave ordering")

            # Short PE warm-up on memset tiles: flips the HAM clock gate to
            # 8/8 while the first DMA wave is still in flight.
            wa = tpool.tile([P, P], DT, tag="warm_a", name="wa")
            wb = tpool.tile([P, 512], DT, tag="warm_b", name="wb")
            nc.vector.memset(wa[:], 0.0)
            nc.vector.memset(wb[:], 0.0)
            pw = psum.tile([P, 512], mybir.dt.float32, tag="pw", name="pw")
            for _ in range(4):
                nc.tensor.matmul(pw, lhsT=wa[:], rhs=wb[:], start=True, stop=True)

            for toff, tsz in tbs:
                # Phase A: hT/vT = w1/w3 @ x for this token block, all d_ff rows
                for fb in range(KF):
                    ph = psum.tile([P, 512], mybir.dt.float32, tag="ph", name="ph")[:, :tsz]
                    pv = psum.tile([P, 512], mybir.dt.float32, tag="pv", name="pv")[:, :tsz]
                    for k in range(KD):
                        nc.tensor.matmul(
                            ph,
                            lhsT=w1_sb[:, k, fb * P : (fb + 1) * P],
                            rhs=x_sb[:, k, toff : toff + tsz],
                            start=(k == 0),
                            stop=(k == KD - 1),
                        )
                    for k in range(KD):
                        nc.tensor.matmul(
                            pv,
                            lhsT=w3_sb[:, k, fb * P : (fb + 1) * P],
                            rhs=x_sb[:, k, toff : toff + tsz],
                            start=(k == 0),
                            stop=(k == KD - 1),
                        )
                    sil = tpool.tile([P, 512], mybir.dt.float32, tag="sil", name="sil")[:, :tsz]
                    nc.scalar.activation(sil, ph, AFT.Silu)
                    nc.vector.tensor_mul(
                        out=y_sb[:, fb, toff : toff + tsz], in0=sil, in1=pv
                    )
                # Phase B: out = y @ w2 for this token block (tokens on partitions)
                for mi in range(tsz // P):
                    mb = toff // P + mi
                    for db in range(D_MODEL // 512):
                        po = psum.tile([P, 512], mybir.dt.float32, tag="po", name="po")
                        for kf in range(KF):
                            nc.tensor.matmul(
                                po,
                                lhsT=y_sb[:, kf, mb * P : (mb + 1) * P],
                                rhs=w2_sb[:, kf, db * 512 : (db + 1) * 512],
                                start=(kf == 0),
                                stop=(kf == KF - 1),
                            )
                        ob = opool.tile([P, 512], mybir.dt.float32, tag="ob", name="ob")
                        nc.vector.tensor_scalar_mul(ob[:], po, rw_sb[:, mb : mb + 1])
                        nc.sync.dma_start(
                            out[mb * P : (mb + 1) * P, db * 512 : (db + 1) * 512], ob
                        )
    _legalize_waits(nc)
    return nc


_BUILD_CACHE = {}


def _get_nc(C):
    if C not in _BUILD_CACHE:
        _BUILD_CACHE[C] = build_ffn(C)
    return _BUILD_CACHE[C]


def _route(xf, gate_w):
    """Top-2 routing (matches jax.lax.top_k + softmax in fp32)."""
    T = xf.shape[0]
    logits = xf @ gate_w.T  # (T, E) fp32
    i1 = np.argmax(logits, axis=1)
    l1 = logits[np.arange(T), i1]
    masked = logits.copy()
    masked[np.arange(T), i1] = -np.inf
    i2 = np.argmax(masked, axis=1)
    l2 = logits[np.arange(T), i2]
    e2 = np.exp((l2 - l1).astype(np.float32))
    rw1 = (1.0 / (1.0 + e2)).astype(np.float32)
    rw2 = (e2 / (1.0 + e2)).astype(np.float32)
    return logits, i1, i2, rw1, rw2


def kernel(x, gate_w, w1, w3, w2, _run_opts=None):
    x = np.ascontiguousarray(np.asarray(x, dtype=np.float32))
    gate_w = np.asarray(gate_w, dtype=np.float32)
    w1 = np.asarray(w1, dtype=np.float32)
    w3 = np.asarray(w3, dtype=np.float32)
    w2 = np.asarray(w2, dtype=np.float32)

    B, S, d = x.shape
    T = B * S
    E = NUM_EXPERTS
    xf = x.reshape(T, d)

    logits, i1, i2, rw1, rw2 = _route(xf, gate_w)

    # Per-expert token lists + shared capacity (multiple of 128)
    sels, rws = [], []
    for e in range(E):
        sel = np.where((i1 == e) | (i2 == e))[0]
        sels.append(sel)
        rws.append(np.where(i1[sel] == e, rw1[sel], rw2[sel]).astype(np.float32))
    C = max(256, -(-max(len(s) for s in sels) // P) * P)

    nc = _get_nc(C)

    in_maps = []
    for e in range(E):
        sel = sels[e]
        n = len(sel)
        xTe = np.zeros((d, C), dtype=BF16)
        xTe[:, :n] = np.ascontiguousarray(xf[sel].T).astype(BF16)
        rwe = np.zeros((C,), dtype=np.float32)
        rwe[:n] = rws[e]
        in_maps.append(
            {
                "xT": xTe,
                "w1T": np.ascontiguousarray(w1[e].T).astype(BF16),
                "w3T": np.ascontiguousarray(w3[e].T).astype(BF16),
                "w2T": np.ascontiguousarray(w2[e].T).astype(BF16),
                "rw": rwe,
            }
        )

    run_opts = _run_opts or {}
    res = run_bass_kernel_spmd(nc, in_maps, core_ids=list(range(N_CORES)), **run_opts)

    outf = np.zeros((T, d), dtype=np.float32)
    for e in range(E):
        sel = sels[e]
        outf[sel] += res.results[e]["out"][: len(sel)]
    output = outf.reshape(B, S, d)

    # Auxiliary load-balance loss (host, fp32 scalar)
    probs = np.exp(logits - logits.max(axis=1, keepdims=True))
    probs /= probs.sum(axis=1, keepdims=True)
    counts = np.bincount(np.concatenate([i1, i2]), minlength=E)
    aux = np.float32(
        (probs.mean(axis=0) * (counts / (T * TOP_K))).sum() * E
    )

    if _run_opts is not None:
        return (output, aux), res
    return output, aux


# revision 8
# speedup vs baseline: 1.0531x; 1.0165x over previous
"""MoE layer (8 experts, top-2 routing, SwiGLU FFN) for 8 Trainium2 NeuronCores.

Sharding strategy (expert-parallel with host-side token dispatch):
  - The router (x @ gate_w.T, top-2, softmax) runs on host as part of computing
    the token dispatch = the sharding of work across cores.
  - Core e receives only the tokens routed to expert e (gathered, padded to a
    common capacity C) plus expert e's weights, all pre-transposed and cast to
    bf16 on host for the device matmul layout.
  - The device kernel computes the expert SwiGLU FFN:
        out = rw * ((silu(x @ w1.T) * (x @ w3.T)) @ w2.T)
    entirely out of SBUF-resident operands (bf16 matmuls, fp32 accumulation).
  - Host scatter-adds the per-expert outputs back into the full (B,S,d) output
    (top-2 => each token's output is the sum of two expert contributions).
  - The auxiliary load-balance loss is a cheap scalar reduction done on host.
"""

import numpy as np
import ml_dtypes

import concourse.bass as bass
import concourse.tile as tile
import concourse.mybir as mybir
from concourse.bass_utils import run_bass_kernel_spmd
from concourse.vector_clock import ScopedClock

BF16 = ml_dtypes.bfloat16
AFT = mybir.ActivationFunctionType

TOP_K = 2
NUM_EXPERTS = 8
D_MODEL = 1024
D_FF = 2048
N_CORES = 8
P = 128

_PATCHED = False


def _patch_drain_wait_split():
    """This walrus build caps sync waits at 1 per instruction (2 for EVSEM),
    but TileContext's final drain can carry one wait per outstanding engine /
    DMA queue.  Split them across individual single-wait sync nops."""
    global _PATCHED
    if _PATCHED:
        return

    def _split_drain_and_barrier(self, tick_clock, wait_clock):
        probe = self.nc.sync.drain()
        wait_clock.add_sem_waits(
            probe.ins, ScopedClock({None: tick_clock.global_clock})
        )
        si = probe.ins.sync_info
        if si is not None and len(si.on_wait) > 1:
            waits = list(si.on_wait)
            probe.ins.sync_info = mybir.SyncInfo(
                on_wait=[waits[0]], on_update=list(si.on_update)
            )
            for w in waits[1:]:
                extra = self.nc.sync.nop(nofuse=True)
                extra.ins.sync_info = mybir.SyncInfo(on_wait=[w], on_update=[])
        self.nc.all_engine_barrier()
        assert self.sems is not None
        popped = self.nc._tile_sem_poison_stack.pop()
        assert popped is self._sem_poison
        self.nc.clear_and_free_semaphores(list(self.sems.allocated().values()))
        self.nc.all_engine_barrier()

    tile.TileContext._drain_and_barrier = _split_drain_and_barrier
    _PATCHED = True


def _legalize_waits(nc):
    """Hardware wait-slot cap: 1 sync wait per instruction (2 for EVSEM).
    Tile's sem assignment can emit more; hoist extras onto single-wait nops
    inserted just before the instruction on the same engine (engines process
    their streams in order, so this preserves semantics)."""
    for fn in nc.m.functions:
        for bb in fn.blocks:
            insts = bb.instructions  # live list
            i = 0
            while i < len(insts):
                inst = insts[i]
                si = inst.sync_info
                cap = 2 if isinstance(inst, mybir.InstEventSemaphore) else 1
                if si is not None and len(si.on_wait) > cap:
                    waits = list(si.on_wait)
                    inst.sync_info = mybir.SyncInfo(
                        on_wait=waits[:cap], on_update=list(si.on_update)
                    )
                    for j, w in enumerate(waits[cap:]):
                        nop = mybir.InstNoOp(
                            name=f"{inst.name}-waitsplit-{j}",
                            sync_info=mybir.SyncInfo(on_wait=[w], on_update=[]),
                            bass_nofuse=True,
                            engine=inst.engine,
                        )
                        insts.insert(i, nop)
                        i += 1
                i += 1


def build_ffn(C):
    """Per-core SwiGLU expert FFN over C gathered tokens.

    DRAM inputs (all bf16 except rw):
      xT  [D_MODEL, C] : gathered tokens, transposed (d on rows)
      w1T [D_MODEL, D_FF], w3T [D_MODEL, D_FF], w2T [D_FF, D_MODEL]
      rw  [C] fp32     : per-token routing weight (0 for padding)
    DRAM output:
      out [C, D_MODEL] fp32 : rw-scaled expert output rows
    """
    _patch_drain_wait_split()
    nc = bass.Bass()
    DT = mybir.dt.bfloat16
    KD = D_MODEL // P   # 8 contraction chunks over d_model
    KF = D_FF // P      # 16 contraction chunks over d_ff
    assert C % 64 == 0
    MB = -(-C // P)     # rw columns (128-token groups, last may be half)

    xT = nc.dram_tensor("xT", [D_MODEL, C], DT, kind="ExternalInput")
    w1T = nc.dram_tensor("w1T", [D_MODEL, D_FF], DT, kind="ExternalInput")
    w3T = nc.dram_tensor("w3T", [D_MODEL, D_FF], DT, kind="ExternalInput")
    w2T = nc.dram_tensor("w2T", [D_FF, D_MODEL], DT, kind="ExternalInput")
    rw = nc.dram_tensor("rw", [MB * P], mybir.dt.float32, kind="ExternalInput")
    out = nc.dram_tensor("out", [C, D_MODEL], mybir.dt.float32, kind="ExternalOutput")

    with tile.TileContext(nc) as tc:
        with (
            tc.tile_pool(name="weights", bufs=1) as wpool,
            tc.tile_pool(name="acts", bufs=1) as apool,
            tc.tile_pool(name="tmp", bufs=4) as tpool,
            tc.tile_pool(name="outp", bufs=4) as opool,
            tc.tile_pool(name="psum", bufs=2, space="PSUM") as psum,
        ):
            # SBUF-resident operands
            x_sb = apool.tile([P, KD, C], DT, tag="x")
            y_sb = apool.tile([P, KF, C], DT, tag="y")
            rw_sb = apool.tile([P, MB], mybir.dt.float32, tag="rw")
            w1_sb = wpool.tile([P, KD, D_FF], DT, tag="w1")
            w3_sb = wpool.tile([P, KD, D_FF], DT, tag="w3")
            w2_sb = wpool.tile([P, KF, D_MODEL], DT, tag="w2")

            xT_r = xT.rearrange("(ko p) c -> p ko c", p=P)
            w1_r = w1T.rearrange("(ko p) f -> p ko f", p=P)
            w3_r = w3T.rearrange("(ko p) f -> p ko f", p=P)
            w2_r = w2T.rearrange("(ko p) d -> p ko d", p=P)

            # token blocks of up to 512 (one PSUM bank of fp32)
            tbs = []
            off = 0
            while off < C:
                sz = min(512, C - off)
                tbs.append((off, sz))
                off += sz

            # DMA waves ordered by consumption: HWDGE queues run concurrently
            # and share HBM bandwidth, so later waves are explicitly gated on
            # earlier ones (add_dep_helper) — the critical first tiles get the
            # full bandwidth and matmuls start ~6us in.
            from concourse.tile_rust import add_dep_helper

            t0off, t0sz = tbs[0]
            waves = []
            waves.append(
                [
                    nc.sync.dma_start(
                        x_sb[:, :, t0off : t0off + t0sz],
                        xT_r[:, :, t0off : t0off + t0sz],
                    ),
                    nc.sync.dma_start(w1_sb[:, :, 0:256], w1_r[:, :, 0:256]),
                    nc.sync.dma_start(w3_sb[:, :, 0:256], w3_r[:, :, 0:256]),
                ]
            )
            for lo, hi in ((256, 768), (768, 1280), (1280, 2048)):
                waves.append(
                    [
                        nc.sync.dma_start(w1_sb[:, :, lo:hi], w1_r[:, :, lo:hi]),
                        nc.sync.dma_start(w3_sb[:, :, lo:hi], w3_r[:, :, lo:hi]),
                    ]
                )
            rest = [
                nc.sync.dma_start(
                    x_sb[:, :, t0off + t0sz :], xT_r[:, :, t0off + t0sz :]
                ),
                nc.sync.dma_start(rw_sb[:], rw.rearrange("(mb p) -> p mb", p=P)),
            ]
            waves.append(rest)
            waves.append(
                [
                    nc.sync.dma_start(w2_sb[:, :KF // 2], w2_r[:, :KF // 2]),
                    nc.sync.dma_start(w2_sb[:, KF // 2 :], w2_r[:, KF // 2 :]),
                ]
            )
            for prev, nxt in zip(waves, waves[1:]):
                for d in nxt:
                    for p_ in prev:
                        add_dep_helper(d.ins, p_.ins, True, "dma wave ordering")

            # Short PE warm-up on memset tiles: flips the HAM clock gate to
            # 8/8 while the first DMA wave is still in flight.
            wa = tpool.tile([P, P], DT, tag="warm_a", name="wa")
            wb = tpool.tile([P, 512], DT, tag="warm_b", name="wb")
            nc.vector.memset(wa[:], 0.0)
            nc.vector.memset(wb[:], 0.0)
            pw = psum.tile([P, 512], mybir.dt.float32, tag="pw", name="pw")
            for _ in range(24):
                nc.tensor.matmul(pw, lhsT=wa[:], rhs=wb[:], start=True, stop=True)

            for toff, tsz in tbs:
                # Phase A: hT/vT = w1/w3 @ x for this token block, all d_ff rows
                for fb in range(KF):
                    ph = psum.tile([P, 512], mybir.dt.float32, tag="ph", name="ph")[:, :tsz]
                    pv = psum.tile([P, 512], mybir.dt.float32, tag="pv", name="pv")[:, :tsz]
                    for k in range(KD):
                        nc.tensor.matmul(
                            ph,
                            lhsT=w1_sb[:, k, fb * P : (fb + 1) * P],
                            rhs=x_sb[:, k, toff : toff + tsz],
                            start=(k == 0),
                            stop=(k == KD - 1),
                        )
                    for k in range(KD):
                        nc.tensor.matmul(
                            pv,
                            lhsT=w3_sb[:, k, fb * P : (fb + 1) * P],
                            rhs=x_sb[:, k, toff : toff + tsz],
                            start=(k == 0),
                            stop=(k == KD - 1),
                        )
                    sil = tpool.tile([P, 512], mybir.dt.float32, tag="sil", name="sil")[:, :tsz]
                    nc.scalar.activation(sil, ph, AFT.Silu)
                    nc.vector.tensor_mul(
                        out=y_sb[:, fb, toff : toff + tsz], in0=sil, in1=pv
                    )
                # Phase B: out = y @ w2 for this token block (tokens on partitions)
                for moff in range(toff, toff + tsz, P):
                    msz = min(P, toff + tsz - moff)
                    mb = moff // P
                    for db in range(D_MODEL // 512):
                        po = psum.tile([P, 512], mybir.dt.float32, tag="po", name="po")[:msz]
                        for kf in range(KF):
                            nc.tensor.matmul(
                                po,
                                lhsT=y_sb[:, kf, moff : moff + msz],
                                rhs=w2_sb[:, kf, db * 512 : (db + 1) * 512],
                                start=(kf == 0),
                                stop=(kf == KF - 1),
                            )
                        ob = opool.tile([P, 512], mybir.dt.float32, tag="ob", name="ob")[:msz]
                        nc.vector.tensor_scalar_mul(ob[:], po, rw_sb[:msz, mb : mb + 1])
                        nc.sync.dma_start(
                            out[moff : moff + msz, db * 512 : (db + 1) * 512], ob
                        )
    _legalize_waits(nc)
    return nc


_BUILD_CACHE = {}


def _get_nc(C):
    if C not in _BUILD_CACHE:
        _BUILD_CACHE[C] = build_ffn(C)
    return _BUILD_CACHE[C]


def _route(xf, gate_w):
    """Top-2 routing (matches jax.lax.top_k + softmax in fp32)."""
    T = xf.shape[0]
    logits = xf @ gate_w.T  # (T, E) fp32
    i1 = np.argmax(logits, axis=1)
    l1 = logits[np.arange(T), i1]
    masked = logits.copy()
    masked[np.arange(T), i1] = -np.inf
    i2 = np.argmax(masked, axis=1)
    l2 = logits[np.arange(T), i2]
    e2 = np.exp((l2 - l1).astype(np.float32))
    rw1 = (1.0 / (1.0 + e2)).astype(np.float32)
    rw2 = (e2 / (1.0 + e2)).astype(np.float32)
    return logits, i1, i2, rw1, rw2


def kernel(x, gate_w, w1, w3, w2, _run_opts=None):
    x = np.ascontiguousarray(np.asarray(x, dtype=np.float32))
    gate_w = np.asarray(gate_w, dtype=np.float32)
    w1 = np.asarray(w1, dtype=np.float32)
    w3 = np.asarray(w3, dtype=np.float32)
    w2 = np.asarray(w2, dtype=np.float32)

    B, S, d = x.shape
    T = B * S
    E = NUM_EXPERTS
    xf = x.reshape(T, d)

    logits, i1, i2, rw1, rw2 = _route(xf, gate_w)

    # Per-expert token lists + shared capacity (multiple of 128)
    sels, rws = [], []
    for e in range(E):
        sel = np.where((i1 == e) | (i2 == e))[0]
        sels.append(sel)
        rws.append(np.where(i1[sel] == e, rw1[sel], rw2[sel]).astype(np.float32))
    C = max(256, -(-max(len(s) for s in sels) // 64) * 64)

    nc = _get_nc(C)

    in_maps = []
    for e in range(E):
        sel = sels[e]
        n = len(sel)
        xTe = np.zeros((d, C), dtype=BF16)
        xTe[:, :n] = np.ascontiguousarray(xf[sel].T).astype(BF16)
        rwe = np.zeros((-(-C // P) * P,), dtype=np.float32)
        rwe[:n] = rws[e]
        in_maps.append(
            {
                "xT": xTe,
                "w1T": np.ascontiguousarray(w1[e].T).astype(BF16),
                "w3T": np.ascontiguousarray(w3[e].T).astype(BF16),
                "w2T": np.ascontiguousarray(w2[e].T).astype(BF16),
                "rw": rwe,
            }
        )

    run_opts = _run_opts or {}
    res = run_bass_kernel_spmd(nc, in_maps, core_ids=list(range(N_CORES)), **run_opts)

    outf = np.zeros((T, d), dtype=np.float32)
    for e in range(E):
        sel = sels[e]
        outf[sel] += res.results[e]["out"][: len(sel)]
    output = outf.reshape(B, S, d)

    # Auxiliary load-balance loss (host, fp32 scalar)
    probs = np.exp(logits - logits.max(axis=1, keepdims=True))
    probs /= probs.sum(axis=1, keepdims=True)
    counts = np.bincount(np.concatenate([i1, i2]), minlength=E)
    aux = np.float32(
        (probs.mean(axis=0) * (counts / (T * TOP_K))).sum() * E
    )

    if _run_opts is not None:
        return (output, aux), res
    return output, aux


# revision 10
# speedup vs baseline: 1.0723x; 1.0182x over previous
"""MoE layer (8 experts, top-2 routing, SwiGLU FFN) for 8 Trainium2 NeuronCores.

Sharding strategy (expert-parallel with host-side token dispatch):
  - The router (x @ gate_w.T, top-2, softmax) runs on host as part of computing
    the token dispatch = the sharding of work across cores.
  - Core e receives only the tokens routed to expert e (gathered, padded to a
    common capacity C) plus expert e's weights, all pre-transposed and cast to
    bf16 on host for the device matmul layout.
  - The device kernel computes the expert SwiGLU FFN:
        out = rw * ((silu(x @ w1.T) * (x @ w3.T)) @ w2.T)
    entirely out of SBUF-resident operands (bf16 matmuls, fp32 accumulation).
  - Host scatter-adds the per-expert outputs back into the full (B,S,d) output
    (top-2 => each token's output is the sum of two expert contributions).
  - The auxiliary load-balance loss is a cheap scalar reduction done on host.
"""

import numpy as np
import ml_dtypes

import concourse.bass as bass
import concourse.tile as tile
import concourse.mybir as mybir
from concourse.bass_utils import run_bass_kernel_spmd
from concourse.vector_clock import ScopedClock

BF16 = ml_dtypes.bfloat16
AFT = mybir.ActivationFunctionType

TOP_K = 2
NUM_EXPERTS = 8
D_MODEL = 1024
D_FF = 2048
N_CORES = 8
P = 128

_PATCHED = False


def _patch_drain_wait_split():
    """This walrus build caps sync waits at 1 per instruction (2 for EVSEM),
    but TileContext's final drain can carry one wait per outstanding engine /
    DMA queue.  Split them across individual single-wait sync nops."""
    global _PATCHED
    if _PATCHED:
        return

    def _split_drain_and_barrier(self, tick_clock, wait_clock):
        probe = self.nc.sync.drain()
        wait_clock.add_sem_waits(
            probe.ins, ScopedClock({None: tick_clock.global_clock})
        )
        si = probe.ins.sync_info
        if si is not None and len(si.on_wait) > 1:
            waits = list(si.on_wait)
            probe.ins.sync_info = mybir.SyncInfo(
                on_wait=[waits[0]], on_update=list(si.on_update)
            )
            for w in waits[1:]:
                extra = self.nc.sync.nop(nofuse=True)
                extra.ins.sync_info = mybir.SyncInfo(on_wait=[w], on_update=[])
        self.nc.all_engine_barrier()
        assert self.sems is not None
        popped = self.nc._tile_sem_poison_stack.pop()
        assert popped is self._sem_poison
        self.nc.clear_and_free_semaphores(list(self.sems.allocated().values()))
        self.nc.all_engine_barrier()

    tile.TileContext._drain_and_barrier = _split_drain_and_barrier
    _PATCHED = True


def _legalize_waits(nc):
    """Hardware wait-slot cap: 1 sync wait per instruction (2 for EVSEM).
    Tile's sem assignment can emit more; hoist extras onto single-wait nops
    inserted just before the instruction on the same engine (engines process
    their streams in order, so this preserves semantics)."""
    for fn in nc.m.functions:
        for bb in fn.blocks:
            insts = bb.instructions  # live list
            i = 0
            while i < len(insts):
                inst = insts[i]
                si = inst.sync_info
                cap = 2 if isinstance(inst, mybir.InstEventSemaphore) else 1
                if si is not None and len(si.on_wait) > cap:
                    waits = list(si.on_wait)
                    inst.sync_info = mybir.SyncInfo(
                        on_wait=waits[:cap], on_update=list(si.on_update)
                    )
                    for j, w in enumerate(waits[cap:]):
                        nop = mybir.InstNoOp(
                            name=f"{inst.name}-waitsplit-{j}",
                            sync_info=mybir.SyncInfo(on_wait=[w], on_update=[]),
                            bass_nofuse=True,
                            engine=inst.engine,
                        )
                        insts.insert(i, nop)
                        i += 1
                i += 1


def build_ffn(C):
    """Per-core SwiGLU expert FFN over C gathered tokens.

    DRAM inputs (all bf16 except rw):
      xT  [D_MODEL, C] : gathered tokens, transposed (d on rows)
      w1T [D_MODEL, D_FF], w3T [D_MODEL, D_FF], w2T [D_FF, D_MODEL]
      rw  [C] fp32     : per-token routing weight (0 for padding)
    DRAM output:
      out [C, D_MODEL] fp32 : rw-scaled expert output rows
    """
    _patch_drain_wait_split()
    nc = bass.Bass()
    DT = mybir.dt.bfloat16
    KD = D_MODEL // P   # 8 contraction chunks over d_model
    KF = D_FF // P      # 16 contraction chunks over d_ff
    assert C % 64 == 0
    MB = -(-C // P)     # rw columns (128-token groups, last may be half)

    xP = nc.dram_tensor("xP", [D_MODEL * C], DT, kind="ExternalInput")
    w1P = nc.dram_tensor("w1P", [D_MODEL * D_FF], DT, kind="ExternalInput")
    w3P = nc.dram_tensor("w3P", [D_MODEL * D_FF], DT, kind="ExternalInput")
    w2T = nc.dram_tensor("w2T", [D_FF, D_MODEL], DT, kind="ExternalInput")
    rw = nc.dram_tensor("rw", [MB * P], mybir.dt.float32, kind="ExternalInput")
    out = nc.dram_tensor("out", [C, D_MODEL], mybir.dt.float32, kind="ExternalOutput")

    with tile.TileContext(nc) as tc:
        with (
            tc.tile_pool(name="weights", bufs=1) as wpool,
            tc.tile_pool(name="acts", bufs=1) as apool,
            tc.tile_pool(name="tmp", bufs=4) as tpool,
            tc.tile_pool(name="outp", bufs=4) as opool,
            tc.tile_pool(name="psum", bufs=2, space="PSUM") as psum,
        ):
            # SBUF-resident operands
            x_sb = apool.tile([P, KD, C], DT, tag="x")
            y_sb = apool.tile([P, KF, C], DT, tag="y")
            rw_sb = apool.tile([P, MB], mybir.dt.float32, tag="rw")
            w1_sb = wpool.tile([P, KD, D_FF], DT, tag="w1")
            w3_sb = wpool.tile([P, KD, D_FF], DT, tag="w3")
            w2_sb = wpool.tile([P, KF, D_MODEL], DT, tag="w2")

            w2_r = w2T.rearrange("(ko p) d -> p ko d", p=P)

            def packed_src(tensor, off, width):
                # wave-contiguous DRAM region [(ko p) width] -> [p ko width]
                return (
                    tensor[off : off + D_MODEL * width]
                    .rearrange("(r c) -> r c", c=width)
                    .rearrange("(ko p) c -> p ko c", p=P)
                )

            # token blocks of up to 512 (one PSUM bank of fp32)
            tbs = []
            off = 0
            while off < C:
                sz = min(512, C - off)
                tbs.append((off, sz))
                off += sz

            # DMA waves ordered by consumption: HWDGE queues run concurrently
            # and share HBM bandwidth, so later waves are explicitly gated on
            # earlier ones (add_dep_helper) — the critical first tiles get the
            # full bandwidth and matmuls start ~6us in.
            from concourse.tile_rust import add_dep_helper

            t0off, t0sz = tbs[0]
            W_SLICES = ((0, 256), (256, 768), (768, 1280), (1280, 2048))
            waves = []
            waves.append(
                [
                    nc.sync.dma_start(
                        x_sb[:, :, t0off : t0off + t0sz],
                        packed_src(xP, 0, t0sz),
                    ),
                    nc.sync.dma_start(
                        w1_sb[:, :, 0:256], packed_src(w1P, 0, 256)
                    ),
                    nc.sync.dma_start(
                        w3_sb[:, :, 0:256], packed_src(w3P, 0, 256)
                    ),
                ]
            )
            for lo, hi in W_SLICES[1:]:
                waves.append(
                    [
                        nc.sync.dma_start(
                            w1_sb[:, :, lo:hi],
                            packed_src(w1P, D_MODEL * lo, hi - lo),
                        ),
                        nc.sync.dma_start(
                            w3_sb[:, :, lo:hi],
                            packed_src(w3P, D_MODEL * lo, hi - lo),
                        ),
                    ]
                )
            rest = [
                nc.sync.dma_start(
                    x_sb[:, :, t0sz:], packed_src(xP, D_MODEL * t0sz, C - t0sz)
                ),
                nc.sync.dma_start(rw_sb[:], rw.rearrange("(mb p) -> p mb", p=P)),
            ]
            waves.append(rest)
            waves.append(
                [
                    nc.sync.dma_start(w2_sb[:, :KF // 2], w2_r[:, :KF // 2]),
                    nc.sync.dma_start(w2_sb[:, KF // 2 :], w2_r[:, KF // 2 :]),
                ]
            )
            for prev, nxt in zip(waves, waves[1:]):
                for d in nxt:
                    for p_ in prev:
                        add_dep_helper(d.ins, p_.ins, True, "dma wave ordering")

            # Short PE warm-up on memset tiles: flips the HAM clock gate to
            # 8/8 while the first DMA wave is still in flight.
            wa = tpool.tile([P, P], DT, tag="warm_a", name="wa")
            wb = tpool.tile([P, 512], DT, tag="warm_b", name="wb")
            nc.vector.memset(wa[:], 0.0)
            nc.vector.memset(wb[:], 0.0)
            pw = psum.tile([P, 512], mybir.dt.float32, tag="pw", name="pw")
            for _ in range(24):
                nc.tensor.matmul(pw, lhsT=wa[:], rhs=wb[:], start=True, stop=True)

            for toff, tsz in tbs:
                # Phase A: hT/vT = w1/w3 @ x for this token block, all d_ff rows
                for fb in range(KF):
                    ph = psum.tile([P, 512], mybir.dt.float32, tag="ph", name="ph")[:, :tsz]
                    pv = psum.tile([P, 512], mybir.dt.float32, tag="pv", name="pv")[:, :tsz]
                    for k in range(KD):
                        nc.tensor.matmul(
                            ph,
                            lhsT=w1_sb[:, k, fb * P : (fb + 1) * P],
                            rhs=x_sb[:, k, toff : toff + tsz],
                            start=(k == 0),
                            stop=(k == KD - 1),
                        )
                    for k in range(KD):
                        nc.tensor.matmul(
                            pv,
                            lhsT=w3_sb[:, k, fb * P : (fb + 1) * P],
                            rhs=x_sb[:, k, toff : toff + tsz],
                            start=(k == 0),
                            stop=(k == KD - 1),
                        )
                    sil = tpool.tile([P, 512], mybir.dt.float32, tag="sil", name="sil")[:, :tsz]
                    nc.scalar.activation(sil, ph, AFT.Silu)
                    nc.vector.tensor_mul(
                        out=y_sb[:, fb, toff : toff + tsz], in0=sil, in1=pv
                    )
                # Phase B: out = y @ w2 for this token block (tokens on partitions)
                for moff in range(toff, toff + tsz, P):
                    msz = min(P, toff + tsz - moff)
                    mb = moff // P
                    for db in range(D_MODEL // 512):
                        po = psum.tile([P, 512], mybir.dt.float32, tag="po", name="po")[:msz]
                        for kf in range(KF):
                            nc.tensor.matmul(
                                po,
                                lhsT=y_sb[:, kf, moff : moff + msz],
                                rhs=w2_sb[:, kf, db * 512 : (db + 1) * 512],
                                start=(kf == 0),
                                stop=(kf == KF - 1),
                            )
                        ob = opool.tile([P, 512], mybir.dt.float32, tag="ob", name="ob")[:msz]
                        nc.vector.tensor_scalar_mul(ob[:], po, rw_sb[:msz, mb : mb + 1])
                        nc.sync.dma_start(
                            out[moff : moff + msz, db * 512 : (db + 1) * 512], ob
                        )
    _legalize_waits(nc)
    return nc


_BUILD_CACHE = {}


def _get_nc(C):
    if C not in _BUILD_CACHE:
        _BUILD_CACHE[C] = build_ffn(C)
    return _BUILD_CACHE[C]


def _route(xf, gate_w):
    """Top-2 routing (matches jax.lax.top_k + softmax in fp32)."""
    T = xf.shape[0]
    logits = xf @ gate_w.T  # (T, E) fp32
    i1 = np.argmax(logits, axis=1)
    l1 = logits[np.arange(T), i1]
    masked = logits.copy()
    masked[np.arange(T), i1] = -np.inf
    i2 = np.argmax(masked, axis=1)
    l2 = logits[np.arange(T), i2]
    e2 = np.exp((l2 - l1).astype(np.float32))
    rw1 = (1.0 / (1.0 + e2)).astype(np.float32)
    rw2 = (e2 / (1.0 + e2)).astype(np.float32)
    return logits, i1, i2, rw1, rw2


def kernel(x, gate_w, w1, w3, w2, _run_opts=None):
    x = np.ascontiguousarray(np.asarray(x, dtype=np.float32))
    gate_w = np.asarray(gate_w, dtype=np.float32)
    w1 = np.asarray(w1, dtype=np.float32)
    w3 = np.asarray(w3, dtype=np.float32)
    w2 = np.asarray(w2, dtype=np.float32)

    B, S, d = x.shape
    T = B * S
    E = NUM_EXPERTS
    xf = x.reshape(T, d)

    logits, i1, i2, rw1, rw2 = _route(xf, gate_w)

    # Per-expert token lists + shared capacity (multiple of 128)
    sels, rws = [], []
    for e in range(E):
        sel = np.where((i1 == e) | (i2 == e))[0]
        sels.append(sel)
        rws.append(np.where(i1[sel] == e, rw1[sel], rw2[sel]).astype(np.float32))
    C = max(256, -(-max(len(s) for s in sels) // 64) * 64)

    nc = _get_nc(C)

    W_SLICES = ((0, 256), (256, 768), (768, 1280), (1280, 2048))
    t0sz = min(512, C)

    def pack_waves(aT, slices):
        # concat of column-slices, each slice row-major (d, width) -> flat
        return np.concatenate(
            [np.ascontiguousarray(aT[:, a:b]).reshape(-1) for a, b in slices]
        )

    in_maps = []
    for e in range(E):
        sel = sels[e]
        n = len(sel)
        xTe = np.zeros((d, C), dtype=BF16)
        xTe[:, :n] = np.ascontiguousarray(xf[sel].T).astype(BF16)
        rwe = np.zeros((-(-C // P) * P,), dtype=np.float32)
        rwe[:n] = rws[e]
        w1Te = np.ascontiguousarray(w1[e].T).astype(BF16)
        w3Te = np.ascontiguousarray(w3[e].T).astype(BF16)
        in_maps.append(
            {
                "xP": pack_waves(xTe, ((0, t0sz), (t0sz, C))),
                "w1P": pack_waves(w1Te, W_SLICES),
                "w3P": pack_waves(w3Te, W_SLICES),
                "w2T": np.ascontiguousarray(w2[e].T).astype(BF16),
                "rw": rwe,
            }
        )

    run_opts = _run_opts or {}
    res = run_bass_kernel_spmd(nc, in_maps, core_ids=list(range(N_CORES)), **run_opts)

    outf = np.zeros((T, d), dtype=np.float32)
    for e in range(E):
        sel = sels[e]
        outf[sel] += res.results[e]["out"][: len(sel)]
    output = outf.reshape(B, S, d)

    # Auxiliary load-balance loss (host, fp32 scalar)
    probs = np.exp(logits - logits.max(axis=1, keepdims=True))
    probs /= probs.sum(axis=1, keepdims=True)
    counts = np.bincount(np.concatenate([i1, i2]), minlength=E)
    aux = np.float32(
        (probs.mean(axis=0) * (counts / (T * TOP_K))).sum() * E
    )

    if _run_opts is not None:
        return (output, aux), res
    return output, aux


# revision 11
# speedup vs baseline: 1.0729x; 1.0006x over previous
"""MoE layer (8 experts, top-2 routing, SwiGLU FFN) for 8 Trainium2 NeuronCores.

Sharding strategy (expert-parallel with host-side token dispatch):
  - The router (x @ gate_w.T, top-2, softmax) runs on host as part of computing
    the token dispatch = the sharding of work across cores.
  - Core e receives only the tokens routed to expert e (gathered, padded to a
    common capacity C) plus expert e's weights, all pre-transposed and cast to
    bf16 on host for the device matmul layout.
  - The device kernel computes the expert SwiGLU FFN:
        out = rw * ((silu(x @ w1.T) * (x @ w3.T)) @ w2.T)
    entirely out of SBUF-resident operands (bf16 matmuls, fp32 accumulation).
  - Host scatter-adds the per-expert outputs back into the full (B,S,d) output
    (top-2 => each token's output is the sum of two expert contributions).
  - The auxiliary load-balance loss is a cheap scalar reduction done on host.
"""

import numpy as np
import ml_dtypes

import concourse.bass as bass
import concourse.tile as tile
import concourse.mybir as mybir
from concourse.bass_utils import run_bass_kernel_spmd
from concourse.vector_clock import ScopedClock

BF16 = ml_dtypes.bfloat16
AFT = mybir.ActivationFunctionType

TOP_K = 2
NUM_EXPERTS = 8
D_MODEL = 1024
D_FF = 2048
N_CORES = 8
P = 128

_PATCHED = False


def _patch_drain_wait_split():
    """This walrus build caps sync waits at 1 per instruction (2 for EVSEM),
    but TileContext's final drain can carry one wait per outstanding engine /
    DMA queue.  Split them across individual single-wait sync nops."""
    global _PATCHED
    if _PATCHED:
        return

    def _split_drain_and_barrier(self, tick_clock, wait_clock):
        probe = self.nc.sync.drain()
        wait_clock.add_sem_waits(
            probe.ins, ScopedClock({None: tick_clock.global_clock})
        )
        si = probe.ins.sync_info
        if si is not None and len(si.on_wait) > 1:
            waits = list(si.on_wait)
            probe.ins.sync_info = mybir.SyncInfo(
                on_wait=[waits[0]], on_update=list(si.on_update)
            )
            for w in waits[1:]:
                extra = self.nc.sync.nop(nofuse=True)
                extra.ins.sync_info = mybir.SyncInfo(on_wait=[w], on_update=[])
        self.nc.all_engine_barrier(sem_only=True)
        assert self.sems is not None
        popped = self.nc._tile_sem_poison_stack.pop()
        assert popped is self._sem_poison
        self.nc.clear_and_free_semaphores(list(self.sems.allocated().values()))
        self.nc.all_engine_barrier(sem_only=True)

    tile.TileContext._drain_and_barrier = _split_drain_and_barrier
    _PATCHED = True


def _legalize_waits(nc):
    """Hardware wait-slot cap: 1 sync wait per instruction (2 for EVSEM).
    Tile's sem assignment can emit more; hoist extras onto single-wait nops
    inserted just before the instruction on the same engine (engines process
    their streams in order, so this preserves semantics)."""
    for fn in nc.m.functions:
        for bb in fn.blocks:
            insts = bb.instructions  # live list
            i = 0
            while i < len(insts):
                inst = insts[i]
                si = inst.sync_info
                cap = 2 if isinstance(inst, mybir.InstEventSemaphore) else 1
                if si is not None and len(si.on_wait) > cap:
                    waits = list(si.on_wait)
                    inst.sync_info = mybir.SyncInfo(
                        on_wait=waits[:cap], on_update=list(si.on_update)
                    )
                    for j, w in enumerate(waits[cap:]):
                        nop = mybir.InstNoOp(
                            name=f"{inst.name}-waitsplit-{j}",
                            sync_info=mybir.SyncInfo(on_wait=[w], on_update=[]),
                            bass_nofuse=True,
                            engine=inst.engine,
                        )
                        insts.insert(i, nop)
                        i += 1
                i += 1


def build_ffn(C):
    """Per-core SwiGLU expert FFN over C gathered tokens.

    DRAM inputs (all bf16 except rw):
      xT  [D_MODEL, C] : gathered tokens, transposed (d on rows)
      w1T [D_MODEL, D_FF], w3T [D_MODEL, D_FF], w2T [D_FF, D_MODEL]
      rw  [C] fp32     : per-token routing weight (0 for padding)
    DRAM output:
      out [C, D_MODEL] fp32 : rw-scaled expert output rows
    """
    _patch_drain_wait_split()
    nc = bass.Bass()
    DT = mybir.dt.bfloat16
    KD = D_MODEL // P   # 8 contraction chunks over d_model
    KF = D_FF // P      # 16 contraction chunks over d_ff
    assert C % 64 == 0
    MB = -(-C // P)     # rw columns (128-token groups, last may be half)

    xP = nc.dram_tensor("xP", [D_MODEL * C], DT, kind="ExternalInput")
    w1P = nc.dram_tensor("w1P", [D_MODEL * D_FF], DT, kind="ExternalInput")
    w3P = nc.dram_tensor("w3P", [D_MODEL * D_FF], DT, kind="ExternalInput")
    w2T = nc.dram_tensor("w2T", [D_FF, D_MODEL], DT, kind="ExternalInput")
    rw = nc.dram_tensor("rw", [MB * P], mybir.dt.float32, kind="ExternalInput")
    out = nc.dram_tensor("out", [C, D_MODEL], mybir.dt.float32, kind="ExternalOutput")

    with tile.TileContext(nc) as tc:
        with (
            tc.tile_pool(name="weights", bufs=1) as wpool,
            tc.tile_pool(name="acts", bufs=1) as apool,
            tc.tile_pool(name="tmp", bufs=4) as tpool,
            tc.tile_pool(name="outp", bufs=4) as opool,
            tc.tile_pool(name="psum", bufs=2, space="PSUM") as psum,
        ):
            # SBUF-resident operands
            x_sb = apool.tile([P, KD, C], DT, tag="x")
            y_sb = apool.tile([P, KF, C], DT, tag="y")
            rw_sb = apool.tile([P, MB], mybir.dt.float32, tag="rw")
            w1_sb = wpool.tile([P, KD, D_FF], DT, tag="w1")
            w3_sb = wpool.tile([P, KD, D_FF], DT, tag="w3")
            w2_sb = wpool.tile([P, KF, D_MODEL], DT, tag="w2")

            w2_r = w2T.rearrange("(ko p) d -> p ko d", p=P)

            def packed_src(tensor, off, width):
                # wave-contiguous DRAM region [(ko p) width] -> [p ko width]
                return (
                    tensor[off : off + D_MODEL * width]
                    .rearrange("(r c) -> r c", c=width)
                    .rearrange("(ko p) c -> p ko c", p=P)
                )

            # token blocks of up to 512 (one PSUM bank of fp32)
            tbs = []
            off = 0
            while off < C:
                sz = min(512, C - off)
                tbs.append((off, sz))
                off += sz

            # DMA waves ordered by consumption: HWDGE queues run concurrently
            # and share HBM bandwidth, so later waves are explicitly gated on
            # earlier ones (add_dep_helper) — the critical first tiles get the
            # full bandwidth and matmuls start ~6us in.
            from concourse.tile_rust import add_dep_helper

            t0off, t0sz = tbs[0]
            W_SLICES = ((0, 256), (256, 768), (768, 1280), (1280, 2048))
            waves = []
            waves.append(
                [
                    nc.sync.dma_start(
                        x_sb[:, :, t0off : t0off + t0sz],
                        packed_src(xP, 0, t0sz),
                    ),
                    nc.sync.dma_start(
                        w1_sb[:, :, 0:256], packed_src(w1P, 0, 256)
                    ),
                    nc.sync.dma_start(
                        w3_sb[:, :, 0:256], packed_src(w3P, 0, 256)
                    ),
                ]
            )
            for lo, hi in W_SLICES[1:]:
                waves.append(
                    [
                        nc.sync.dma_start(
                            w1_sb[:, :, lo:hi],
                            packed_src(w1P, D_MODEL * lo, hi - lo),
                        ),
                        nc.sync.dma_start(
                            w3_sb[:, :, lo:hi],
                            packed_src(w3P, D_MODEL * lo, hi - lo),
                        ),
                    ]
                )
            rest = [
                nc.sync.dma_start(
                    x_sb[:, :, t0sz:], packed_src(xP, D_MODEL * t0sz, C - t0sz)
                ),
                nc.sync.dma_start(rw_sb[:], rw.rearrange("(mb p) -> p mb", p=P)),
            ]
            waves.append(rest)
            waves.append(
                [
                    nc.sync.dma_start(w2_sb[:, :KF // 2], w2_r[:, :KF // 2]),
                    nc.sync.dma_start(w2_sb[:, KF // 2 :], w2_r[:, KF // 2 :]),
                ]
            )
            for prev, nxt in zip(waves, waves[1:]):
                for d in nxt:
                    for p_ in prev:
                        add_dep_helper(d.ins, p_.ins, True, "dma wave ordering")

            # Short PE warm-up on memset tiles: flips the HAM clock gate to
            # 8/8 while the first DMA wave is still in flight.
            wa = tpool.tile([P, P], DT, tag="warm_a", name="wa")
            wb = tpool.tile([P, 512], DT, tag="warm_b", name="wb")
            nc.vector.memset(wa[:], 0.0)
            nc.vector.memset(wb[:], 0.0)
            pw = psum.tile([P, 512], mybir.dt.float32, tag="pw", name="pw")
            for _ in range(24):
                nc.tensor.matmul(pw, lhsT=wa[:], rhs=wb[:], start=True, stop=True)

            for toff, tsz in tbs:
                # Phase A: hT/vT = w1/w3 @ x for this token block, all d_ff rows
                for fb in range(KF):
                    ph = psum.tile([P, 512], mybir.dt.float32, tag="ph", name="ph")[:, :tsz]
                    pv = psum.tile([P, 512], mybir.dt.float32, tag="pv", name="pv")[:, :tsz]
                    for k in range(KD):
                        nc.tensor.matmul(
                            ph,
                            lhsT=w1_sb[:, k, fb * P : (fb + 1) * P],
                            rhs=x_sb[:, k, toff : toff + tsz],
                            start=(k == 0),
                            stop=(k == KD - 1),
                        )
                    for k in range(KD):
                        nc.tensor.matmul(
                            pv,
                            lhsT=w3_sb[:, k, fb * P : (fb + 1) * P],
                            rhs=x_sb[:, k, toff : toff + tsz],
                            start=(k == 0),
                            stop=(k == KD - 1),
                        )
                    sil = tpool.tile([P, 512], mybir.dt.float32, tag="sil", name="sil")[:, :tsz]
                    nc.scalar.activation(sil, ph, AFT.Silu)
                    nc.vector.tensor_mul(
                        out=y_sb[:, fb, toff : toff + tsz], in0=sil, in1=pv
                    )
                # Phase B: out = y @ w2 for this token block (tokens on partitions)
                for moff in range(toff, toff + tsz, P):
                    msz = min(P, toff + tsz - moff)
                    mb = moff // P
                    for db in range(D_MODEL // 512):
                        po = psum.tile([P, 512], mybir.dt.float32, tag="po", name="po")[:msz]
                        for kf in range(KF):
                            nc.tensor.matmul(
                                po,
                                lhsT=y_sb[:, kf, moff : moff + msz],
                                rhs=w2_sb[:, kf, db * 512 : (db + 1) * 512],
                                start=(kf == 0),
                                stop=(kf == KF - 1),
                            )
                        ob = opool.tile([P, 512], mybir.dt.float32, tag="ob", name="ob")[:msz]
                        nc.vector.tensor_scalar_mul(ob[:], po, rw_sb[:msz, mb : mb + 1])
                        nc.sync.dma_start(
                            out[moff : moff + msz, db * 512 : (db + 1) * 512], ob
                        )
    _legalize_waits(nc)
    return nc


_BUILD_CACHE = {}


def _get_nc(C):
    if C not in _BUILD_CACHE:
        _BUILD_CACHE[C] = build_ffn(C)
    return _BUILD_CACHE[C]


def _route(xf, gate_w):
    """Top-2 routing (matches jax.lax.top_k + softmax in fp32)."""
    T = xf.shape[0]
    logits = xf @ gate_w.T  # (T, E) fp32
    i1 = np.argmax(logits, axis=1)
    l1 = logits[np.arange(T), i1]
    masked = logits.copy()
    masked[np.arange(T), i1] = -np.inf
    i2 = np.argmax(masked, axis=1)
    l2 = logits[np.arange(T), i2]
    e2 = np.exp((l2 - l1).astype(np.float32))
    rw1 = (1.0 / (1.0 + e2)).astype(np.float32)
    rw2 = (e2 / (1.0 + e2)).astype(np.float32)
    return logits, i1, i2, rw1, rw2


def kernel(x, gate_w, w1, w3, w2, _run_opts=None):
    x = np.ascontiguousarray(np.asarray(x, dtype=np.float32))
    gate_w = np.asarray(gate_w, dtype=np.float32)
    w1 = np.asarray(w1, dtype=np.float32)
    w3 = np.asarray(w3, dtype=np.float32)
    w2 = np.asarray(w2, dtype=np.float32)

    B, S, d = x.shape
    T = B * S
    E = NUM_EXPERTS
    xf = x.reshape(T, d)

    logits, i1, i2, rw1, rw2 = _route(xf, gate_w)

    # Per-expert token lists + shared capacity (multiple of 128)
    sels, rws = [], []
    for e in range(E):
        sel = np.where((i1 == e) | (i2 == e))[0]
        sels.append(sel)
        rws.append(np.where(i1[sel] == e, rw1[sel], rw2[sel]).astype(np.float32))
    C = max(256, -(-max(len(s) for s in sels) // 64) * 64)

    nc = _get_nc(C)

    W_SLICES = ((0, 256), (256, 768), (768, 1280), (1280, 2048))
    t0sz = min(512, C)

    def pack_waves(aT, slices):
        # concat of column-slices, each slice row-major (d, width) -> flat
        return np.concatenate(
            [np.ascontiguousarray(aT[:, a:b]).reshape(-1) for a, b in slices]
        )

    in_maps = []
    for e in range(E):
        sel = sels[e]
        n = len(sel)
        xTe = np.zeros((d, C), dtype=BF16)
        xTe[:, :n] = np.ascontiguousarray(xf[sel].T).astype(BF16)
        rwe = np.zeros((-(-C // P) * P,), dtype=np.float32)
        rwe[:n] = rws[e]
        w1Te = np.ascontiguousarray(w1[e].T).astype(BF16)
        w3Te = np.ascontiguousarray(w3[e].T).astype(BF16)
        in_maps.append(
            {
                "xP": pack_waves(xTe, ((0, t0sz), (t0sz, C))),
                "w1P": pack_waves(w1Te, W_SLICES),
                "w3P": pack_waves(w3Te, W_SLICES),
                "w2T": np.ascontiguousarray(w2[e].T).astype(BF16),
                "rw": rwe,
            }
        )

    run_opts = _run_opts or {}
    res = run_bass_kernel_spmd(nc, in_maps, core_ids=list(range(N_CORES)), **run_opts)

    outf = np.zeros((T, d), dtype=np.float32)
    for e in range(E):
        sel = sels[e]
        outf[sel] += res.results[e]["out"][: len(sel)]
    output = outf.reshape(B, S, d)

    # Auxiliary load-balance loss (host, fp32 scalar)
    probs = np.exp(logits - logits.max(axis=1, keepdims=True))
    probs /= probs.sum(axis=1, keepdims=True)
    counts = np.bincount(np.concatenate([i1, i2]), minlength=E)
    aux = np.float32(
        (probs.mean(axis=0) * (counts / (T * TOP_K))).sum() * E
    )

    if _run_opts is not None:
        return (output, aux), res
    return output, aux


# revision 12
# speedup vs baseline: 1.0744x; 1.0014x over previous
"""MoE layer (8 experts, top-2 routing, SwiGLU FFN) for 8 Trainium2 NeuronCores.

Sharding strategy (expert-parallel with host-side token dispatch):
  - The router (x @ gate_w.T, top-2, softmax) runs on host as part of computing
    the token dispatch = the sharding of work across cores.
  - Core e receives only the tokens routed to expert e (gathered, padded to a
    common capacity C) plus expert e's weights, all pre-transposed and cast to
    bf16 on host for the device matmul layout.
  - The device kernel computes the expert SwiGLU FFN:
        out = rw * ((silu(x @ w1.T) * (x @ w3.T)) @ w2.T)
    entirely out of SBUF-resident operands (bf16 matmuls, fp32 accumulation).
  - Host scatter-adds the per-expert outputs back into the full (B,S,d) output
    (top-2 => each token's output is the sum of two expert contributions).
  - The auxiliary load-balance loss is a cheap scalar reduction done on host.
"""

import numpy as np
import ml_dtypes

import concourse.bass as bass
import concourse.tile as tile
import concourse.mybir as mybir
from concourse.bass_utils import run_bass_kernel_spmd
from concourse.vector_clock import ScopedClock

BF16 = ml_dtypes.bfloat16
AFT = mybir.ActivationFunctionType

TOP_K = 2
NUM_EXPERTS = 8
D_MODEL = 1024
D_FF = 2048
N_CORES = 8
P = 128

_PATCHED = False


def _patch_drain_wait_split():
    """This walrus build caps sync waits at 1 per instruction (2 for EVSEM),
    but TileContext's final drain can carry one wait per outstanding engine /
    DMA queue.  Split them across individual single-wait sync nops."""
    global _PATCHED
    if _PATCHED:
        return

    def _split_drain_and_barrier(self, tick_clock, wait_clock):
        import os
        if os.environ.get("SKIP_SEM_CLEANUP"):
            self._skip_sem_cleanup = True
        probe = self.nc.sync.drain()
        wait_clock.add_sem_waits(
            probe.ins, ScopedClock({None: tick_clock.global_clock})
        )
        si = probe.ins.sync_info
        if si is not None and len(si.on_wait) > 1:
            waits = list(si.on_wait)
            probe.ins.sync_info = mybir.SyncInfo(
                on_wait=[waits[0]], on_update=list(si.on_update)
            )
            for w in waits[1:]:
                extra = self.nc.sync.nop(nofuse=True)
                extra.ins.sync_info = mybir.SyncInfo(on_wait=[w], on_update=[])
        self.nc.all_engine_barrier(sem_only=True)
        assert self.sems is not None
        popped = self.nc._tile_sem_poison_stack.pop()
        assert popped is self._sem_poison
        if not getattr(self, "_skip_sem_cleanup", False):
            self.nc.clear_and_free_semaphores(list(self.sems.allocated().values()))
        self.nc.all_engine_barrier(sem_only=True)

    tile.TileContext._drain_and_barrier = _split_drain_and_barrier
    _PATCHED = True


def _legalize_waits(nc):
    """Hardware wait-slot cap: 1 sync wait per instruction (2 for EVSEM).
    Tile's sem assignment can emit more; hoist extras onto single-wait nops
    inserted just before the instruction on the same engine (engines process
    their streams in order, so this preserves semantics)."""
    for fn in nc.m.functions:
        for bb in fn.blocks:
            insts = bb.instructions  # live list
            i = 0
            while i < len(insts):
                inst = insts[i]
                si = inst.sync_info
                cap = 2 if isinstance(inst, mybir.InstEventSemaphore) else 1
                if si is not None and len(si.on_wait) > cap:
                    waits = list(si.on_wait)
                    inst.sync_info = mybir.SyncInfo(
                        on_wait=waits[:cap], on_update=list(si.on_update)
                    )
                    for j, w in enumerate(waits[cap:]):
                        nop = mybir.InstNoOp(
                            name=f"{inst.name}-waitsplit-{j}",
                            sync_info=mybir.SyncInfo(on_wait=[w], on_update=[]),
                            bass_nofuse=True,
                            engine=inst.engine,
                        )
                        insts.insert(i, nop)
                        i += 1
                i += 1


def build_ffn(C):
    """Per-core SwiGLU expert FFN over C gathered tokens.

    DRAM inputs (all bf16 except rw):
      xT  [D_MODEL, C] : gathered tokens, transposed (d on rows)
      w1T [D_MODEL, D_FF], w3T [D_MODEL, D_FF], w2T [D_FF, D_MODEL]
      rw  [C] fp32     : per-token routing weight (0 for padding)
    DRAM output:
      out [C, D_MODEL] fp32 : rw-scaled expert output rows
    """
    _patch_drain_wait_split()
    nc = bass.Bass()
    DT = mybir.dt.bfloat16
    KD = D_MODEL // P   # 8 contraction chunks over d_model
    KF = D_FF // P      # 16 contraction chunks over d_ff
    assert C % 64 == 0
    MB = -(-C // P)     # rw columns (128-token groups, last may be half)

    xP = nc.dram_tensor("xP", [D_MODEL * C], DT, kind="ExternalInput")
    w1P = nc.dram_tensor("w1P", [D_MODEL * D_FF], DT, kind="ExternalInput")
    w3P = nc.dram_tensor("w3P", [D_MODEL * D_FF], DT, kind="ExternalInput")
    w2T = nc.dram_tensor("w2T", [D_FF, D_MODEL], DT, kind="ExternalInput")
    rw = nc.dram_tensor("rw", [MB * P], mybir.dt.float32, kind="ExternalInput")
    out = nc.dram_tensor("out", [C, D_MODEL], mybir.dt.float32, kind="ExternalOutput")

    with tile.TileContext(nc) as tc:
        with (
            tc.tile_pool(name="weights", bufs=1) as wpool,
            tc.tile_pool(name="acts", bufs=1) as apool,
            tc.tile_pool(name="tmp", bufs=4) as tpool,
            tc.tile_pool(name="outp", bufs=4) as opool,
            tc.tile_pool(name="psum", bufs=2, space="PSUM") as psum,
        ):
            # SBUF-resident operands
            x_sb = apool.tile([P, KD, C], DT, tag="x")
            y_sb = apool.tile([P, KF, C], DT, tag="y")
            rw_sb = apool.tile([P, MB], mybir.dt.float32, tag="rw")
            w1_sb = wpool.tile([P, KD, D_FF], DT, tag="w1")
            w3_sb = wpool.tile([P, KD, D_FF], DT, tag="w3")
            w2_sb = wpool.tile([P, KF, D_MODEL], DT, tag="w2")

            w2_r = w2T.rearrange("(ko p) d -> p ko d", p=P)

            def packed_src(tensor, off, width):
                # wave-contiguous DRAM region [(ko p) width] -> [p ko width]
                return (
                    tensor[off : off + D_MODEL * width]
                    .rearrange("(r c) -> r c", c=width)
                    .rearrange("(ko p) c -> p ko c", p=P)
                )

            # token blocks of up to 512 (one PSUM bank of fp32)
            tbs = []
            off = 0
            while off < C:
                sz = min(512, C - off)
                tbs.append((off, sz))
                off += sz

            # DMA waves ordered by consumption: HWDGE queues run concurrently
            # and share HBM bandwidth, so later waves are explicitly gated on
            # earlier ones (add_dep_helper) — the critical first tiles get the
            # full bandwidth and matmuls start ~6us in.
            from concourse.tile_rust import add_dep_helper

            t0off, t0sz = tbs[0]
            W_SLICES = ((0, 256), (256, 768), (768, 1280), (1280, 2048))
            waves = []
            waves.append(
                [
                    nc.sync.dma_start(
                        x_sb[:, :, t0off : t0off + t0sz],
                        packed_src(xP, 0, t0sz),
                    ),
                    nc.sync.dma_start(
                        w1_sb[:, :, 0:256], packed_src(w1P, 0, 256)
                    ),
                    nc.sync.dma_start(
                        w3_sb[:, :, 0:256], packed_src(w3P, 0, 256)
                    ),
                ]
            )
            for lo, hi in W_SLICES[1:]:
                waves.append(
                    [
                        nc.sync.dma_start(
                            w1_sb[:, :, lo:hi],
                            packed_src(w1P, D_MODEL * lo, hi - lo),
                        ),
                        nc.sync.dma_start(
                            w3_sb[:, :, lo:hi],
                            packed_src(w3P, D_MODEL * lo, hi - lo),
                        ),
                    ]
                )
            rest = [
                nc.sync.dma_start(
                    x_sb[:, :, t0sz:], packed_src(xP, D_MODEL * t0sz, C - t0sz)
                ),
                nc.sync.dma_start(rw_sb[:], rw.rearrange("(mb p) -> p mb", p=P)),
            ]
            waves.append(rest)
            waves.append(
                [
                    nc.sync.dma_start(w2_sb[:, :KF // 2], w2_r[:, :KF // 2]),
                    nc.sync.dma_start(w2_sb[:, KF // 2 :], w2_r[:, KF // 2 :]),
                ]
            )
            for prev, nxt in zip(waves, waves[1:]):
                for d in nxt:
                    for p_ in prev:
                        add_dep_helper(d.ins, p_.ins, True, "dma wave ordering")

            # Short PE warm-up on memset tiles: flips the HAM clock gate to
            # 8/8 while the first DMA wave is still in flight.
            wa = tpool.tile([P, P], DT, tag="warm_a", name="wa")
            wb = tpool.tile([P, 512], DT, tag="warm_b", name="wb")
            nc.vector.memset(wa[:], 0.0)
            nc.vector.memset(wb[:], 0.0)
            pw = psum.tile([P, 512], mybir.dt.float32, tag="pw", name="pw")
            for _ in range(24):
                nc.tensor.matmul(pw, lhsT=wa[:], rhs=wb[:], start=True, stop=True)

            for toff, tsz in tbs:
                # Phase A: hT/vT = w1/w3 @ x for this token block, all d_ff rows
                for fb in range(KF):
                    ph = psum.tile([P, 512], mybir.dt.float32, tag="ph", name="ph")[:, :tsz]
                    pv = psum.tile([P, 512], mybir.dt.float32, tag="pv", name="pv")[:, :tsz]
                    for k in range(KD):
                        nc.tensor.matmul(
                            ph,
                            lhsT=w1_sb[:, k, fb * P : (fb + 1) * P],
                            rhs=x_sb[:, k, toff : toff + tsz],
                            start=(k == 0),
                            stop=(k == KD - 1),
                        )
                    for k in range(KD):
                        nc.tensor.matmul(
                            pv,
                            lhsT=w3_sb[:, k, fb * P : (fb + 1) * P],
                            rhs=x_sb[:, k, toff : toff + tsz],
                            start=(k == 0),
                            stop=(k == KD - 1),
                        )
                    sil = tpool.tile([P, 512], mybir.dt.float32, tag="sil", name="sil")[:, :tsz]
                    nc.scalar.activation(sil, ph, AFT.Silu)
                    nc.vector.tensor_mul(
                        out=y_sb[:, fb, toff : toff + tsz], in0=sil, in1=pv
                    )
                # Phase B: out = y @ w2 for this token block (tokens on partitions)
                for moff in range(toff, toff + tsz, P):
                    msz = min(P, toff + tsz - moff)
                    mb = moff // P
                    for db in range(D_MODEL // 512):
                        po = psum.tile([P, 512], mybir.dt.float32, tag="po", name="po")[:msz]
                        for kf in range(KF):
                            nc.tensor.matmul(
                                po,
                                lhsT=y_sb[:, kf, moff : moff + msz],
                                rhs=w2_sb[:, kf, db * 512 : (db + 1) * 512],
                                start=(kf == 0),
                                stop=(kf == KF - 1),
                            )
                        ob = opool.tile([P, 512], mybir.dt.float32, tag="ob", name="ob")[:msz]
                        nc.vector.tensor_scalar_mul(ob[:], po, rw_sb[:msz, mb : mb + 1])
                        nc.sync.dma_start(
                            out[moff : moff + msz, db * 512 : (db + 1) * 512], ob
                        )
    _legalize_waits(nc)
    return nc


_BUILD_CACHE = {}


def _get_nc(C):
    if C not in _BUILD_CACHE:
        _BUILD_CACHE[C] = build_ffn(C)
    return _BUILD_CACHE[C]


def _route(xf, gate_w):
    """Top-2 routing (matches jax.lax.top_k + softmax in fp32)."""
    T = xf.shape[0]
    logits = xf @ gate_w.T  # (T, E) fp32
    i1 = np.argmax(logits, axis=1)
    l1 = logits[np.arange(T), i1]
    masked = logits.copy()
    masked[np.arange(T), i1] = -np.inf
    i2 = np.argmax(masked, axis=1)
    l2 = logits[np.arange(T), i2]
    e2 = np.exp((l2 - l1).astype(np.float32))
    rw1 = (1.0 / (1.0 + e2)).astype(np.float32)
    rw2 = (e2 / (1.0 + e2)).astype(np.float32)
    return logits, i1, i2, rw1, rw2


def kernel(x, gate_w, w1, w3, w2, _run_opts=None):
    x = np.ascontiguousarray(np.asarray(x, dtype=np.float32))
    gate_w = np.asarray(gate_w, dtype=np.float32)
    w1 = np.asarray(w1, dtype=np.float32)
    w3 = np.asarray(w3, dtype=np.float32)
    w2 = np.asarray(w2, dtype=np.float32)

    B, S, d = x.shape
    T = B * S
    E = NUM_EXPERTS
    xf = x.reshape(T, d)

    logits, i1, i2, rw1, rw2 = _route(xf, gate_w)

    # Per-expert token lists + shared capacity (multiple of 128)
    sels, rws = [], []
    for e in range(E):
        sel = np.where((i1 == e) | (i2 == e))[0]
        sels.append(sel)
        rws.append(np.where(i1[sel] == e, rw1[sel], rw2[sel]).astype(np.float32))
    C = max(256, -(-max(len(s) for s in sels) // 64) * 64)

    nc = _get_nc(C)

    W_SLICES = ((0, 256), (256, 768), (768, 1280), (1280, 2048))
    t0sz = min(512, C)

    def pack_waves(aT, slices):
        # concat of column-slices, each slice row-major (d, width) -> flat
        return np.concatenate(
            [np.ascontiguousarray(aT[:, a:b]).reshape(-1) for a, b in slices]
        )

    in_maps = []
    for e in range(E):
        sel = sels[e]
        n = len(sel)
        xTe = np.zeros((d, C), dtype=BF16)
        xTe[:, :n] = np.ascontiguousarray(xf[sel].T).astype(BF16)
        rwe = np.zeros((-(-C // P) * P,), dtype=np.float32)
        rwe[:n] = rws[e]
        w1Te = np.ascontiguousarray(w1[e].T).astype(BF16)
        w3Te = np.ascontiguousarray(w3[e].T).astype(BF16)
        in_maps.append(
            {
                "xP": pack_waves(xTe, ((0, t0sz), (t0sz, C))),
                "w1P": pack_waves(w1Te, W_SLICES),
                "w3P": pack_waves(w3Te, W_SLICES),
                "w2T": np.ascontiguousarray(w2[e].T).astype(BF16),
                "rw": rwe,
            }
        )

    run_opts = _run_opts or {}
    res = run_bass_kernel_spmd(nc, in_maps, core_ids=list(range(N_CORES)), **run_opts)

    outf = np.zeros((T, d), dtype=np.float32)
    for e in range(E):
        sel = sels[e]
        outf[sel] += res.results[e]["out"][: len(sel)]
    output = outf.reshape(B, S, d)

    # Auxiliary load-balance loss (host, fp32 scalar)
    probs = np.exp(logits - logits.max(axis=1, keepdims=True))
    probs /= probs.sum(axis=1, keepdims=True)
    counts = np.bincount(np.concatenate([i1, i2]), minlength=E)
    aux = np.float32(
        (probs.mean(axis=0) * (counts / (T * TOP_K))).sum() * E
    )

    if _run_opts is not None:
        return (output, aux), res
    return output, aux


# revision 13
# speedup vs baseline: 1.0765x; 1.0020x over previous
"""MoE layer (8 experts, top-2 routing, SwiGLU FFN) for 8 Trainium2 NeuronCores.

Sharding strategy (expert-parallel with host-side token dispatch):
  - The router (x @ gate_w.T, top-2, softmax) runs on host as part of computing
    the token dispatch = the sharding of work across cores.
  - Core e receives only the tokens routed to expert e (gathered, padded to a
    common capacity C) plus expert e's weights, all pre-transposed and cast to
    bf16 on host for the device matmul layout.
  - The device kernel computes the expert SwiGLU FFN:
        out = rw * ((silu(x @ w1.T) * (x @ w3.T)) @ w2.T)
    entirely out of SBUF-resident operands (bf16 matmuls, fp32 accumulation).
  - Host scatter-adds the per-expert outputs back into the full (B,S,d) output
    (top-2 => each token's output is the sum of two expert contributions).
  - The auxiliary load-balance loss is a cheap scalar reduction done on host.
"""

import numpy as np
import ml_dtypes

import concourse.bass as bass
import concourse.tile as tile
import concourse.mybir as mybir
from concourse.bass_utils import run_bass_kernel_spmd
from concourse.vector_clock import ScopedClock

BF16 = ml_dtypes.bfloat16
AFT = mybir.ActivationFunctionType

TOP_K = 2
NUM_EXPERTS = 8
D_MODEL = 1024
D_FF = 2048
N_CORES = 8
P = 128

_PATCHED = False


def _patch_drain_wait_split():
    """This walrus build caps sync waits at 1 per instruction (2 for EVSEM),
    but TileContext's final drain can carry one wait per outstanding engine /
    DMA queue.  Split them across individual single-wait sync nops."""
    global _PATCHED
    if _PATCHED:
        return

    def _split_drain_and_barrier(self, tick_clock, wait_clock):
        import os
        if os.environ.get("SKIP_SEM_CLEANUP"):
            self._skip_sem_cleanup = True
        probe = self.nc.sync.drain()
        wait_clock.add_sem_waits(
            probe.ins, ScopedClock({None: tick_clock.global_clock})
        )
        si = probe.ins.sync_info
        if si is not None and len(si.on_wait) > 1:
            waits = list(si.on_wait)
            probe.ins.sync_info = mybir.SyncInfo(
                on_wait=[waits[0]], on_update=list(si.on_update)
            )
            for w in waits[1:]:
                extra = self.nc.sync.nop(nofuse=True)
                extra.ins.sync_info = mybir.SyncInfo(on_wait=[w], on_update=[])
        self.nc.all_engine_barrier(sem_only=True)
        assert self.sems is not None
        popped = self.nc._tile_sem_poison_stack.pop()
        assert popped is self._sem_poison
        if not getattr(self, "_skip_sem_cleanup", False):
            self.nc.clear_and_free_semaphores(list(self.sems.allocated().values()))
        self.nc.all_engine_barrier(sem_only=True)

    tile.TileContext._drain_and_barrier = _split_drain_and_barrier
    _PATCHED = True


def _legalize_waits(nc):
    """Hardware wait-slot cap: 1 sync wait per instruction (2 for EVSEM).
    Tile's sem assignment can emit more; hoist extras onto single-wait nops
    inserted just before the instruction on the same engine (engines process
    their streams in order, so this preserves semantics)."""
    for fn in nc.m.functions:
        for bb in fn.blocks:
            insts = bb.instructions  # live list
            i = 0
            while i < len(insts):
                inst = insts[i]
                si = inst.sync_info
                cap = 2 if isinstance(inst, mybir.InstEventSemaphore) else 1
                if si is not None and len(si.on_wait) > cap:
                    waits = list(si.on_wait)
                    inst.sync_info = mybir.SyncInfo(
                        on_wait=waits[:cap], on_update=list(si.on_update)
                    )
                    for j, w in enumerate(waits[cap:]):
                        nop = mybir.InstNoOp(
                            name=f"{inst.name}-waitsplit-{j}",
                            sync_info=mybir.SyncInfo(on_wait=[w], on_update=[]),
                            bass_nofuse=True,
                            engine=inst.engine,
                        )
                        insts.insert(i, nop)
                        i += 1
                i += 1


def build_ffn(C):
    """Per-core SwiGLU expert FFN over C gathered tokens.

    DRAM inputs (all bf16 except rw):
      xT  [D_MODEL, C] : gathered tokens, transposed (d on rows)
      w1T [D_MODEL, D_FF], w3T [D_MODEL, D_FF], w2T [D_FF, D_MODEL]
      rw  [C] fp32     : per-token routing weight (0 for padding)
    DRAM output:
      out [C, D_MODEL] fp32 : rw-scaled expert output rows
    """
    _patch_drain_wait_split()
    nc = bass.Bass()
    DT = mybir.dt.bfloat16
    KD = D_MODEL // P   # 8 contraction chunks over d_model
    KF = D_FF // P      # 16 contraction chunks over d_ff
    assert C % 64 == 0
    MB = -(-C // P)     # rw columns (128-token groups, last may be half)

    xP = nc.dram_tensor("xP", [D_MODEL * C], DT, kind="ExternalInput")
    w1P = nc.dram_tensor("w1P", [D_MODEL * D_FF], DT, kind="ExternalInput")
    w3P = nc.dram_tensor("w3P", [D_MODEL * D_FF], DT, kind="ExternalInput")
    w2T = nc.dram_tensor("w2T", [D_FF, D_MODEL], DT, kind="ExternalInput")
    rw = nc.dram_tensor("rw", [MB * P], mybir.dt.float32, kind="ExternalInput")
    out = nc.dram_tensor("out", [C, D_MODEL], mybir.dt.float32, kind="ExternalOutput")

    with tile.TileContext(nc) as tc:
        with (
            tc.tile_pool(name="weights", bufs=1) as wpool,
            tc.tile_pool(name="acts", bufs=1) as apool,
            tc.tile_pool(name="tmp", bufs=4) as tpool,
            tc.tile_pool(name="outp", bufs=4) as opool,
            tc.tile_pool(name="psum", bufs=2, space="PSUM") as psum,
        ):
            # SBUF-resident operands
            x_sb = apool.tile([P, KD, C], DT, tag="x")
            y_sb = apool.tile([P, KF, C], DT, tag="y")
            rw_sb = apool.tile([P, MB], mybir.dt.float32, tag="rw")
            w1_sb = wpool.tile([P, KD, D_FF], DT, tag="w1")
            w3_sb = wpool.tile([P, KD, D_FF], DT, tag="w3")
            w2_sb = wpool.tile([P, KF, D_MODEL], DT, tag="w2")

            w2_r = w2T.rearrange("(ko p) d -> p ko d", p=P)

            def packed_src(tensor, off, width):
                # wave-contiguous DRAM region [(ko p) width] -> [p ko width]
                return (
                    tensor[off : off + D_MODEL * width]
                    .rearrange("(r c) -> r c", c=width)
                    .rearrange("(ko p) c -> p ko c", p=P)
                )

            # token blocks of up to 512 (one PSUM bank of fp32)
            tbs = []
            off = 0
            while off < C:
                sz = min(512, C - off)
                tbs.append((off, sz))
                off += sz

            # DMA waves ordered by consumption: HWDGE queues run concurrently
            # and share HBM bandwidth, so later waves are explicitly gated on
            # earlier ones (add_dep_helper) — the critical first tiles get the
            # full bandwidth and matmuls start ~6us in.
            from concourse.tile_rust import add_dep_helper

            t0off, t0sz = tbs[0]
            W_SLICES = ((0, 256), (256, 768), (768, 1280), (1280, 2048))
            waves = []
            waves.append(
                [
                    nc.sync.dma_start(
                        x_sb[:, :, t0off : t0off + t0sz],
                        packed_src(xP, 0, t0sz),
                    ),
                    nc.sync.dma_start(
                        w1_sb[:, :, 0:256], packed_src(w1P, 0, 256)
                    ),
                    nc.sync.dma_start(
                        w3_sb[:, :, 0:256], packed_src(w3P, 0, 256)
                    ),
                ]
            )
            for lo, hi in W_SLICES[1:]:
                waves.append(
                    [
                        nc.sync.dma_start(
                            w1_sb[:, :, lo:hi],
                            packed_src(w1P, D_MODEL * lo, hi - lo),
                        ),
                        nc.sync.dma_start(
                            w3_sb[:, :, lo:hi],
                            packed_src(w3P, D_MODEL * lo, hi - lo),
                        ),
                    ]
                )
            rest = [
                nc.sync.dma_start(
                    x_sb[:, :, t0sz:], packed_src(xP, D_MODEL * t0sz, C - t0sz)
                ),
                nc.sync.dma_start(rw_sb[:], rw.rearrange("(mb p) -> p mb", p=P)),
            ]
            waves.append(rest)
            waves.append(
                [
                    nc.sync.dma_start(w2_sb[:, :KF // 2], w2_r[:, :KF // 2]),
                    nc.sync.dma_start(w2_sb[:, KF // 2 :], w2_r[:, KF // 2 :]),
                ]
            )
            for prev, nxt in zip(waves, waves[1:]):
                for d in nxt:
                    for p_ in prev:
                        add_dep_helper(d.ins, p_.ins, True, "dma wave ordering")

            # Short PE warm-up on memset tiles: flips the HAM clock gate to
            # 8/8 while the first DMA wave is still in flight.
            wa = tpool.tile([P, P], DT, tag="warm_a", name="wa")
            wb = tpool.tile([P, 512], DT, tag="warm_b", name="wb")
            nc.vector.memset(wa[:], 0.0)
            nc.vector.memset(wb[:], 0.0)
            pw = psum.tile([P, 512], mybir.dt.float32, tag="pw", name="pw")
            for _ in range(18):
                nc.tensor.matmul(pw, lhsT=wa[:], rhs=wb[:], start=True, stop=True)

            for toff, tsz in tbs:
                # Phase A: hT/vT = w1/w3 @ x for this token block, all d_ff rows
                for fb in range(KF):
                    ph = psum.tile([P, 512], mybir.dt.float32, tag="ph", name="ph")[:, :tsz]
                    pv = psum.tile([P, 512], mybir.dt.float32, tag="pv", name="pv")[:, :tsz]
                    for k in range(KD):
                        nc.tensor.matmul(
                            ph,
                            lhsT=w1_sb[:, k, fb * P : (fb + 1) * P],
                            rhs=x_sb[:, k, toff : toff + tsz],
                            start=(k == 0),
                            stop=(k == KD - 1),
                        )
                    for k in range(KD):
                        nc.tensor.matmul(
                            pv,
                            lhsT=w3_sb[:, k, fb * P : (fb + 1) * P],
                            rhs=x_sb[:, k, toff : toff + tsz],
                            start=(k == 0),
                            stop=(k == KD - 1),
                        )
                    sil = tpool.tile([P, 512], mybir.dt.float32, tag="sil", name="sil")[:, :tsz]
                    nc.scalar.activation(sil, ph, AFT.Silu)
                    nc.vector.tensor_mul(
                        out=y_sb[:, fb, toff : toff + tsz], in0=sil, in1=pv
                    )
                # Phase B: out = y @ w2 for this token block (tokens on partitions)
                for moff in range(toff, toff + tsz, P):
                    msz = min(P, toff + tsz - moff)
                    mb = moff // P
                    for db in range(D_MODEL // 512):
                        po = psum.tile([P, 512], mybir.dt.float32, tag="po", name="po")[:msz]
                        for kf in range(KF):
                            nc.tensor.matmul(
                                po,
                                lhsT=y_sb[:, kf, moff : moff + msz],
                                rhs=w2_sb[:, kf, db * 512 : (db + 1) * 512],
                                start=(kf == 0),
                                stop=(kf == KF - 1),
                            )
                        ob = opool.tile([P, 512], mybir.dt.float32, tag="ob", name="ob")[:msz]
                        nc.vector.tensor_scalar_mul(ob[:], po, rw_sb[:msz, mb : mb + 1])
                        nc.sync.dma_start(
                            out[moff : moff + msz, db * 512 : (db + 1) * 512], ob
                        )
    _legalize_waits(nc)
    return nc


_BUILD_CACHE = {}


def _get_nc(C):
    if C not in _BUILD_CACHE:
        _BUILD_CACHE[C] = build_ffn(C)
    return _BUILD_CACHE[C]


def _route(xf, gate_w):
    """Top-2 routing (matches jax.lax.top_k + softmax in fp32)."""
    T = xf.shape[0]
    logits = xf @ gate_w.T  # (T, E) fp32
    i1 = np.argmax(logits, axis=1)
    l1 = logits[np.arange(T), i1]
    masked = logits.copy()
    masked[np.arange(T), i1] = -np.inf
    i2 = np.argmax(masked, axis=1)
    l2 = logits[np.arange(T), i2]
    e2 = np.exp((l2 - l1).astype(np.float32))
    rw1 = (1.0 / (1.0 + e2)).astype(np.float32)
    rw2 = (e2 / (1.0 + e2)).astype(np.float32)
    return logits, i1, i2, rw1, rw2


def kernel(x, gate_w, w1, w3, w2, _run_opts=None):
    x = np.ascontiguousarray(np.asarray(x, dtype=np.float32))
    gate_w = np.asarray(gate_w, dtype=np.float32)
    w1 = np.asarray(w1, dtype=np.float32)
    w3 = np.asarray(w3, dtype=np.float32)
    w2 = np.asarray(w2, dtype=np.float32)

    B, S, d = x.shape
    T = B * S
    E = NUM_EXPERTS
    xf = x.reshape(T, d)

    logits, i1, i2, rw1, rw2 = _route(xf, gate_w)

    # Per-expert token lists + shared capacity (multiple of 128)
    sels, rws = [], []
    for e in range(E):
        sel = np.where((i1 == e) | (i2 == e))[0]
        sels.append(sel)
        rws.append(np.where(i1[sel] == e, rw1[sel], rw2[sel]).astype(np.float32))
    C = max(256, -(-max(len(s) for s in sels) // 64) * 64)

    nc = _get_nc(C)

    W_SLICES = ((0, 256), (256, 768), (768, 1280), (1280, 2048))
    t0sz = min(512, C)

    def pack_waves(aT, slices):
        # concat of column-slices, each slice row-major (d, width) -> flat
        return np.concatenate(
            [np.ascontiguousarray(aT[:, a:b]).reshape(-1) for a, b in slices]
        )

    in_maps = []
    for e in range(E):
        sel = sels[e]
        n = len(sel)
        xTe = np.zeros((d, C), dtype=BF16)
        xTe[:, :n] = np.ascontiguousarray(xf[sel].T).astype(BF16)
        rwe = np.zeros((-(-C // P) * P,), dtype=np.float32)
        rwe[:n] = rws[e]
        w1Te = np.ascontiguousarray(w1[e].T).astype(BF16)
        w3Te = np.ascontiguousarray(w3[e].T).astype(BF16)
        in_maps.append(
            {
                "xP": pack_waves(xTe, ((0, t0sz), (t0sz, C))),
                "w1P": pack_waves(w1Te, W_SLICES),
                "w3P": pack_waves(w3Te, W_SLICES),
                "w2T": np.ascontiguousarray(w2[e].T).astype(BF16),
                "rw": rwe,
            }
        )

    run_opts = _run_opts or {}
    res = run_bass_kernel_spmd(nc, in_maps, core_ids=list(range(N_CORES)), **run_opts)

    outf = np.zeros((T, d), dtype=np.float32)
    for e in range(E):
        sel = sels[e]
        outf[sel] += res.results[e]["out"][: len(sel)]
    output = outf.reshape(B, S, d)

    # Auxiliary load-balance loss (host, fp32 scalar)
    probs = np.exp(logits - logits.max(axis=1, keepdims=True))
    probs /= probs.sum(axis=1, keepdims=True)
    counts = np.bincount(np.concatenate([i1, i2]), minlength=E)
    aux = np.float32(
        (probs.mean(axis=0) * (counts / (T * TOP_K))).sum() * E
    )

    if _run_opts is not None:
        return (output, aux), res
    return output, aux
